# revision 22
# baseline (speedup 1.0000x reference)
"""Trainium2 Bass kernel for nn_DSLDSCell (moe_routing).

Data-parallel over 8 NeuronCores: each core processes 512 of the 4096 rows.
Heavy per-row k-weighted reductions (transition/Q/Ab contra k_new) run on the
TensorEngine as 2-rows-per-matmul block-diagonal contractions (k=64 on
partitions; 2 consecutive rows fill 128 partitions and are DRAM-contiguous).
Per-row 16x16 factorization uses LDL^T (no sqrt in the recurrence),
vectorized across 128 partitions. LayerNorm rsqrt is DVE-Newton (bit-trick
seed), keeping the ACT engine on only two table sets (gelu / ln+exp).
"""
import math
import sys

for _p in ("/opt/trn_rl_repo",):
    if _p not in sys.path:
        sys.path.insert(0, _p)

import numpy as np

import concourse.bass as bass
import concourse.bacc as bacc
import concourse.tile as tile
from concourse import mybir
from concourse.masks import make_identity

f32 = mybir.dt.float32
i32 = mybir.dt.int32
OP = mybir.AluOpType
AF = mybir.ActivationFunctionType
AX = mybir.AxisListType

N, D, K, X, H = 4096, 16, 64, 128, 256
NCORES = 8
NC = N // NCORES          # rows per core (512)
P = 128                   # partitions
NG = NC // P              # groups per core (4)
NPAIR = P // 2            # pairs per group (64)
NB = D + 2                # rhs columns in the triangular solve
LOG2PI = math.log(2.0 * math.pi)
MAGIC = 0x5F3759DF

f16 = mybir.dt.float16
GELU = "act"                # "act" = ACT Gelu_apprx_tanh; "tanh" = composite
                            # (CoreSim does not implement Gelu)

_PROGRAM = None


# --------------------------------------------------------------------------
# device program
# --------------------------------------------------------------------------

def _patch_ldw_opt():
    """The default walrus invocation passes --enable-ldw-opt=false; our
    stream matmuls are LDWEIGHTS-bound (fp16, 128-column weights), so
    re-enable the fast-weight-load path for this kernel's compiles."""
    import concourse.bass_utils as _bu
    if getattr(_bu, "_ldw_patched", False):
        return
    _orig = _bu.run_command

    def patched(cmd, **kw):
        cmd = ["--enable-ldw-opt=true" if c == "--enable-ldw-opt=false" else c
               for c in cmd]
        return _orig(cmd, **kw)

    _bu.run_command = patched
    _bu._ldw_patched = True


def _patch_act_tables():
    """Force walrus's table-load pass to place Exp/Ln/Square in
    natural_log_exp_and_others and Gelu_apprx_tanh in its own set, so the
    kernel only ever loads two ACT table sets (indices stay file-aligned)."""
    import concourse.bacc as _bacc
    if getattr(_bacc, "_act_tables_patched", False):
        return
    _orig = _bacc.get_activation_tables

    def patched(arch):
        t = _orig(arch)
        keep = {"natural_log_exp_and_others", "gelu_apprx_tanh_and_others"}
        drop = {AF.Exp, AF.Ln, AF.Square, AF.Gelu_apprx_tanh, AF.Tanh}
        for name, fns in t.items():
            if name not in keep:
                for f in drop:
                    fns.discard(f)
        return t

    _bacc.get_activation_tables = patched
    _bacc._act_tables_patched = True


def build_program():
    _patch_act_tables()
    nc = bacc.Bacc("TRN2", debug=False, num_devices=NCORES)

    dd = {}
    def din(name, shape, dt=f32):
        dd[name] = nc.declare_dram_parameter(name, shape, dt, isOutput=False)
    def dout(name, shape):
        dd[name] = nc.declare_dram_parameter(name, shape, f32, isOutput=True)

    din("z", [NC, D]); din("ks", [NC, K]); din("xt", [NC, X])
    din("maskf", [NC, 1], i32); din("gn", [NC, K]); din("eps", [NC, D])
    din("trans", [NG, P, NPAIR, K], f16)
    din("Qp", [NG, P, NPAIR, D * D], f16)
    din("Abp", [NG, P, NPAIR, D * 17], f16)
    din("fzw0", [D + K + X, H]); din("fzw1", [H, H]); din("fzw2", [H, K])
    din("ezw0", [2 * X + K, H]); din("ezw1", [H, H]); din("ezw2", [H, 2 * D])
    din("wra", [K * D + K, X]); din("qzs", [1, D]); din("tempv", [1, 1])
    dout("o_sample", [NC, D]); dout("o_sc", [NC, 4]); dout("o_qk", [NC, K])

    with tile.TileContext(nc) as tc:
        _emit(nc, tc, dd)
    nc.compile()
    return nc, ["o_sample", "o_sc", "o_qk"]


def _bc1(ap, mid, inner):
    """[P, inner] AP -> [P, mid, inner] with zero-stride middle dim."""
    return ap.rearrange("p (a n) -> p a n", a=1).broadcast_to(
        (ap.shape[0], mid, inner))


def _bci(ap, nk, inner):
    """[P, nk] AP -> [P, nk, inner] with zero-stride inner dim."""
    return ap.rearrange("p (k o) -> p k o", o=1).broadcast_to(
        (ap.shape[0], nk, inner))


def _diag(t, n, stride, off=0):
    """[P, n] view of diagonal-ish elements: offset + i*stride."""
    return bass.AP(tensor=t.tensor, offset=t.offset + off,
                   ap=[t.ap[0], [stride, n]])


def _emit(nc, tc, d):
    import contextlib
    ctx = contextlib.ExitStack()
    consts = ctx.enter_context(tc.tile_pool(name="consts", bufs=1))
    work = ctx.enter_context(tc.tile_pool(name="work", bufs=NG + 1))
    scratch = ctx.enter_context(tc.tile_pool(name="scratch", bufs=2))
    stream = ctx.enter_context(tc.tile_pool(name="stream", bufs=2))
    pmm = ctx.enter_context(tc.tile_pool(name="pmm", bufs=2, space="PSUM"))
    ptp = ctx.enter_context(tc.tile_pool(name="ptp", bufs=2, space="PSUM"))
    pstr = ctx.enter_context(tc.tile_pool(name="pstr", bufs=1, space="PSUM"))

    V, T = nc.vector, nc.tensor
    GP = nc.gpsimd
    STT = nc.vector.scalar_tensor_tensor
    _last_act = [None]

    class _SWrap:
        """Chain ACT ops in emission order so the scheduler cannot
        interleave activation-table sets across phases."""
        @staticmethod
        def activation(*a, **kw):
            bi = nc.scalar.activation(*a, **kw)
            if _last_act[0] is not None:
                bass._add_dep_helper(bi.ins, _last_act[0], sync=False,
                                     reason="act-table-order")
            _last_act[0] = bi.ins
            return bi
    S = _SWrap

    # ---------------- constants ----------------
    ident = consts.tile([P, P], f32)
    make_identity(nc, ident)
    magic16 = consts.tile([P, 16], i32)
    V.memset(magic16, MAGIC)
    zero_c = consts.tile([P, P], f32)
    V.memset(zero_c, 0.0)

    def newton_rsqrt(dst, y, tagp, iters=3):
        """dst = 1/sqrt(y); y, dst [P, w] f32 contiguous, w <= 16."""
        w = y.shape[-1]
        hv = scratch.tile([P, w], i32, tag=tagp + "_h", name=tagp + "_h")
        V.tensor_scalar(out=hv, in0=y.bitcast(i32), scalar1=1, scalar2=None,
                        op0=OP.logical_shift_right)
        V.tensor_tensor(out=dst.bitcast(i32), in0=magic16[:, 0:w], in1=hv,
                        op=OP.subtract)
        t = scratch.tile([P, w], f32, tag=tagp + "_t", name=tagp + "_t")
        e = scratch.tile([P, w], f32, tag=tagp + "_e", name=tagp + "_e")
        for _ in range(iters):
            if w == 1:
                STT(out=t, in0=dst, scalar=dst, in1=y, op0=OP.mult, op1=OP.mult)
            else:
                V.tensor_tensor(out=t, in0=dst, in1=dst, op=OP.mult)
                V.tensor_tensor(out=t, in0=t, in1=y, op=OP.mult)
            V.tensor_scalar(out=e, in0=t, scalar1=-0.5, scalar2=1.5,
                            op0=OP.mult, op1=OP.add)
            V.tensor_tensor(out=dst, in0=dst, in1=e, op=OP.mult)

    tempb = consts.tile([P, 1], f32)
    nc.gpsimd.dma_start(out=tempb, in_=bass.AP(
        tensor=d["tempv"], offset=0, ap=[[0, P], [1, 1]]))
    invt = consts.tile([P, 1], f32)
    V.reciprocal(out=invt, in_=tempb)
    ntmp = consts.tile([P, 1], f32)
    V.tensor_scalar(out=ntmp, in0=tempb, scalar1=-1.0, scalar2=None,
                    op0=OP.mult)

    qzs_b = consts.tile([P, D], f32)
    nc.gpsimd.dma_start(out=qzs_b, in_=bass.AP(
        tensor=d["qzs"], offset=0, ap=[[0, P], [1, D]]))
    qzs_cl = consts.tile([P, D], f32)
    V.tensor_scalar(out=qzs_cl, in0=qzs_b, scalar1=-3.0, scalar2=None,
                    op0=OP.max)
    qsig = consts.tile([P, D], f32)
    S.activation(out=qsig, in_=qzs_cl, func=AF.Exp)
    rsig = consts.tile([P, D], f32)
    V.reciprocal(out=rsig, in_=qsig)
    slsig = consts.tile([P, 1], f32)
    V.tensor_reduce(out=slsig, in_=qzs_cl, axis=AX.X, op=OP.add)

    def load_w(name, dn, do):
        chunks = []
        for c in range((dn + P - 1) // P):
            cl = min(P, dn - c * P)
            t = consts.tile([P, do], f32, tag=f"{name}{c}", name=f"{name}{c}")
            nc.gpsimd.dma_start(out=t[:cl, :], in_=d[name][c * P:c * P + cl, :])
            chunks.append((t, cl))
        return chunks

    fzw0 = load_w("fzw0", D + K + X, H)
    fzw1 = load_w("fzw1", H, H)
    fzw2 = load_w("fzw2", H, K)
    ezw0 = load_w("ezw0", 2 * X + K, H)
    ezw1 = load_w("ezw1", H, H)
    ezw2 = load_w("ezw2", H, 2 * D)
    wra = load_w("wra", K * D + K, X)

    def mm(out, lhsT, rhs, **kw):
        T.matmul(out, lhsT, rhs, **kw)

    def layernorm(x, out, tagp):
        st = scratch.tile([P, 6], f32, tag=tagp + "_st", name=tagp + "_st")
        V.bn_stats(out=st, in_=x)
        mv = scratch.tile([P, 2], f32, tag=tagp + "_mv", name=tagp + "_mv")
        V.bn_aggr(out=mv, in_=st)
        ve = scratch.tile([P, 1], f32, tag=tagp + "_ve", name=tagp + "_ve")
        V.tensor_scalar(out=ve, in0=mv[:, 1:2], scalar1=1e-6, scalar2=None,
                        op0=OP.add)
        rst = scratch.tile([P, 1], f32, tag=tagp + "_rs", name=tagp + "_rs")
        newton_rsqrt(rst, ve, tagp)
        V.tensor_scalar(out=out, in0=x, scalar1=mv[:, 0:1], scalar2=rst,
                        op0=OP.subtract, op1=OP.mult)

    def dense(x, wchunks, do, gelu, out_sb, tagp, psum_lo=0):
        """out_sb = [gelu](x @ W)[:, psum_lo:psum_lo+width(out_sb)]."""
        xts = []
        for c, (w_, cl) in enumerate(wchunks):
            pt = ptp.tile([P, P], f32, tag="ptp", name="ptp")
            T.transpose(pt[:cl, :], x[:, c * P:c * P + cl], ident)
            xT = scratch.tile([P, P], f32, tag=tagp + f"_xT{c}", name=tagp + f"_xT{c}")
            nc.scalar.copy(out=xT[:cl, :], in_=pt[:cl, :])
            xts.append(xT)
        ph = pmm.tile([P, 512], f32, tag="pmm", name="pmm")
        nchunk = len(wchunks)
        for c, (w_, cl) in enumerate(wchunks):
            mm(ph[:, 0:do], xts[c][:cl, :], w_[:cl, 0:do],
               start=(c == 0), stop=(c == nchunk - 1))
        wo = out_sb.shape[-1]
        if gelu and GELU == "act":
            S.activation(out=out_sb, in_=ph[:, psum_lo:psum_lo + wo],
                         func=AF.Gelu_apprx_tanh)
        elif gelu:
            xs = scratch.tile([P, wo], f32, tag=tagp + "_gx", name=tagp + "_gx")
            V.tensor_copy(out=xs, in_=ph[:, psum_lo:psum_lo + wo])
            t3 = scratch.tile([P, wo], f32, tag=tagp + "_g3", name=tagp + "_g3")
            V.tensor_tensor(out=t3, in0=xs, in1=xs, op=OP.mult)
            V.tensor_tensor(out=t3, in0=t3, in1=xs, op=OP.mult)
            STT(out=t3, in0=t3, scalar=0.044715, in1=xs, op0=OP.mult, op1=OP.add)
            S.activation(out=t3, in_=t3, func=AF.Tanh,
                         scale=0.7978845608028654)
            V.tensor_scalar(out=t3, in0=t3, scalar1=0.5, scalar2=0.5,
                            op0=OP.mult, op1=OP.add)
            V.tensor_tensor(out=out_sb, in0=t3, in1=xs, op=OP.mult)
        else:
            V.tensor_copy(out=out_sb, in_=ph[:, psum_lo:psum_lo + wo])

    def blockdiag_lhsT(src64, tag):
        """src64: [P, K] rows tile -> transpose -> [128,128] block-diag
        columns: col 2t+0 = src row 2t on parts 0:64, col 2t+1 = row 2t+1
        on parts 64:128."""
        pt = ptp.tile([P, P], f32, tag="ptp", name="ptp")
        T.transpose(pt[0:K, :], src64, ident)
        sT = scratch.tile([K, P], f16, tag=tag + "_T", name=tag + "_T")
        nc.scalar.copy(out=sT, in_=pt[0:K, :])
        LA = scratch.tile([P, P], f16, tag=tag + "_LA", name=tag + "_LA")
        GP.memset(LA, 0.0)
        GP.tensor_copy(
            out=LA[0:K, :].rearrange("p (t two) -> p t two", two=2)[:, :, 0:1],
            in_=sT.rearrange("p (t two) -> p t two", two=2)[:, :, 0:1])
        GP.tensor_copy(
            out=LA[K:P, :].rearrange("p (t two) -> p t two", two=2)[:, :, 1:2],
            in_=sT.rearrange("p (t two) -> p t two", two=2)[:, :, 1:2])
        return LA

    G = {}

    def wt(shape, tag):
        return work.tile(shape, f32, tag=tag, name=tag)

    # ============ PHASE A: inputs, fz net (gelu set), transition MMs ========
    for g in range(NG):
        r0 = g * P
        st = {}
        G[g] = st
        in0 = wt([P, D + K + X], "in0")
        nc.gpsimd.dma_start(out=in0[:, 0:D], in_=d["z"][r0:r0 + P, :])
        nc.gpsimd.dma_start(out=in0[:, D:D + K], in_=d["ks"][r0:r0 + P, :])
        nc.gpsimd.dma_start(out=in0[:, D + K:], in_=d["xt"][r0:r0 + P, :])
        st["in0"] = in0
        mfk = work.tile([P, 1], i32, tag="mfk", name="mfk")
        nc.gpsimd.dma_start(out=mfk, in_=d["maskf"][r0:r0 + P, :])
        st["mfk"] = mfk
        gnt = wt([P, K], "gnt")
        nc.gpsimd.dma_start(out=gnt, in_=d["gn"][r0:r0 + P, :])
        st["gnt"] = gnt
        ept = wt([P, D], "ept")
        nc.gpsimd.dma_start(out=ept, in_=d["eps"][r0:r0 + P, :])
        st["ept"] = ept

        # ---- fz net ----
        xh = scratch.tile([P, D + K + X], f32, tag="fz_xh", name="fz_xh")
        layernorm(in0, xh, "fzl0")
        h0 = scratch.tile([P, H], f32, tag="fz_h0", name="fz_h0")
        dense(xh, fzw0, H, True, h0, "fzd0")
        xh1 = scratch.tile([P, H], f32, tag="fz_xh1", name="fz_xh1")
        layernorm(h0, xh1, "fzl1")
        h1 = scratch.tile([P, H], f32, tag="fz_h1", name="fz_h1")
        dense(xh1, fzw1, H, True, h1, "fzd1")
        xh2 = scratch.tile([P, H], f32, tag="fz_xh2", name="fz_xh2")
        layernorm(h1, xh2, "fzl2")
        qk = wt([P, K], "qk")
        dense(xh2, fzw2, K, False, qk, "fzd2")
        st["qk"] = qk

        # ---- transition pair-MMs (data stationary, kn-blockdiag moving) ----
        # out[j, r] per pair -> psum_tT [64, 2t+r]; transpose back afterwards.
        LK = blockdiag_lhsT(in0[:, D:D + K], "kf")
        pairbase = g * NPAIR
        ptT = pstr.tile([K, P], f32, tag="ptT", name="ptT")
        for ch in range(4):                       # 4 chunks of 16 pairs
            trt = stream.tile([P, 16, K], f16, tag="trch", name="trch")
            nc.sync.dma_start(
                out=trt, in_=d["trans"][g, :, ch * 16:(ch + 1) * 16, :])
            for i in range(16):
                pr = ch * 16 + i
                mm(ptT[:, 2 * pr:2 * pr + 2], trt[:, i, :],
                   LK[:, 2 * pr:2 * pr + 2], start=True, stop=True)
        tT_sb = scratch.tile([K, P], f32, tag="tT_sb", name="tT_sb")
        nc.scalar.copy(out=tT_sb, in_=ptT)
        ptb = ptp.tile([P, P], f32, tag="ptp", name="ptp")
        T.transpose(ptb[:, 0:K], tT_sb, ident[0:K, 0:K])
        pkp = wt([P, K], "pkp")
        V.tensor_copy(out=pkp, in_=ptb[:, 0:K])
        st["pkp"] = pkp

    # ============ PHASE B: ln/exp set — pk_logits, softmaxes, k_new, d_iwae =
    for g in range(NG):
        st = G[g]
        pkl = scratch.tile([P, K], f32, tag="pkl", name="pkl")
        S.activation(out=pkl, in_=st["pkp"], func=AF.Ln)

        gt_n = scratch.tile([P, K], f32, tag="gt_n", name="gt_n")
        V.tensor_scalar(out=gt_n, in0=st["gnt"], scalar1=invt[:, 0:1],
                        scalar2=None, op0=OP.mult)

        def softmax_t(logits, tagp, out_tile):
            sx = scratch.tile([P, K], f32, tag=tagp + "_sx", name=tagp + "_sx")
            STT(out=sx, in0=logits, scalar=invt[:, 0:1], in1=gt_n,
                op0=OP.mult, op1=OP.add)
            nm = scratch.tile([P, 1], f32, tag=tagp + "_nm", name=tagp + "_nm")
            V.tensor_reduce(out=nm, in_=sx, axis=AX.X, op=OP.max, negate=True)
            ex = scratch.tile([P, K], f32, tag=tagp + "_ex", name=tagp + "_ex")
            ssum = scratch.tile([P, 1], f32, tag=tagp + "_ss", name=tagp + "_ss")
            S.activation(out=ex, in_=sx, func=AF.Exp, bias=nm, accum_out=ssum)
            rs = scratch.tile([P, 1], f32, tag=tagp + "_rs", name=tagp + "_rs")
            V.reciprocal(out=rs, in_=ssum)
            V.tensor_scalar(out=out_tile, in0=ex, scalar1=rs, scalar2=None,
                            op0=OP.mult)

        qks = scratch.tile([P, K], f32, tag="qks", name="qks")
        softmax_t(st["qk"], "smq", qks)
        pks = scratch.tile([P, K], f32, tag="pks", name="pks")
        softmax_t(pkl, "smp", pks)

        kn = wt([P, K], "kn")
        V.tensor_copy(out=kn, in_=pks)
        V.copy_predicated(out=kn, mask=st["mfk"][:, 0:1].broadcast_to((P, K)),
                          data=qks)
        st["kn"] = kn

        logx = scratch.tile([P, K], f32, tag="logx", name="logx")
        S.activation(out=logx, in_=kn, func=AF.Ln)

        def lse(logits, tagp, out_t):
            sc = scratch.tile([P, K], f32, tag=tagp + "_sc", name=tagp + "_sc")
            STT(out=sc, in0=logx, scalar=ntmp[:, 0:1], in1=logits,
                op0=OP.mult, op1=OP.add)
            nm = scratch.tile([P, 1], f32, tag=tagp + "_nm", name=tagp + "_nm")
            V.tensor_reduce(out=nm, in_=sc, axis=AX.X, op=OP.max, negate=True)
            ex = scratch.tile([P, K], f32, tag=tagp + "_ex", name=tagp + "_ex")
            sm = scratch.tile([P, 1], f32, tag=tagp + "_sm", name=tagp + "_sm")
            S.activation(out=ex, in_=sc, func=AF.Exp, bias=nm, accum_out=sm)
            ls = scratch.tile([P, 1], f32, tag=tagp + "_ls", name=tagp + "_ls")
            S.activation(out=ls, in_=sm, func=AF.Ln)
            V.tensor_tensor(out=out_t, in0=ls, in1=nm, op=OP.subtract)

        lq = scratch.tile([P, 1], f32, tag="lseq_o", name="lseq_o")
        lse(st["qk"], "lseq", lq)
        lp = scratch.tile([P, 1], f32, tag="lsep_o", name="lsep_o")
        lse(pkl, "lsep", lp)

        df = scratch.tile([P, K], f32, tag="df", name="df")
        V.tensor_tensor(out=df, in0=st["qk"], in1=pkl, op=OP.subtract)
        dsum = scratch.tile([P, 1], f32, tag="dsum", name="dsum")
        V.tensor_reduce(out=dsum, in_=df, axis=AX.X, op=OP.add)
        dl = scratch.tile([P, 1], f32, tag="dl", name="dl")
        V.tensor_tensor(out=dl, in0=lq, in1=lp, op=OP.subtract)
        diw = wt([P, 1], "diw")
        STT(out=diw, in0=dl, scalar=-float(K), in1=dsum,
            op0=OP.mult, op1=OP.add)
        st["diw"] = diw

    # ============ PHASE C: streams, gt, ez net (gelu), LDLT, solves =========
    for g in range(NG):
        st = G[g]
        in0 = st["in0"]
        kn = st["kn"]
        zt = in0[:, 0:D]
        pairbase = g * NPAIR

        LN_ = blockdiag_lhsT(kn, "kn")

        # ---- Q stream: out[de, r] per pair; psum_qT [128, 4t+2h+r] ----
        Qk = wt([P, D * D], "Qk")
        pqT = pstr.tile([P, 4 * NPAIR], f32, tag="pqT", name="pqT")
        for ch in range(4):
            qt = stream.tile([P, 16, D * D], f16, tag="qch", name="qch")
            nc.sync.dma_start(
                out=qt, in_=d["Qp"][g, :, ch * 16:(ch + 1) * 16, :])
            for i in range(16):
                pr = ch * 16 + i
                for h in range(2):
                    mm(pqT[:, 4 * pr + 2 * h:4 * pr + 2 * h + 2],
                       qt[:, i, h * P:(h + 1) * P],
                       LN_[:, 2 * pr:2 * pr + 2], start=True, stop=True)
        qT_sb = scratch.tile([P, 4 * NPAIR], f32, tag="qT_sb", name="qT_sb")
        V.tensor_copy(
            out=qT_sb.rearrange("p (h t r) -> p h t r", h=2, r=2),
            in_=pqT.rearrange("p (t h r) -> p h t r", h=2, r=2))
        for h in range(2):
            ptb = ptp.tile([P, P], f32, tag="ptp", name="ptp")
            T.transpose(ptb[:, :], qT_sb[:, h * P:(h + 1) * P], ident)
            V.tensor_copy(out=Qk[:, h * P:(h + 1) * P], in_=ptb[:, :])

        # ---- Ab stream: 3 de-chunks (128,128,16); psum_aT [128, 6t+2h+r] ----
        Abk = wt([P, D * 17], "Abk")
        paT = pstr.tile([P, 6 * NPAIR], f32, tag="paT", name="paT")
        for ch in range(4):
            at = stream.tile([P, 16, D * 17], f16, tag="abch", name="abch")
            nc.sync.dma_start(
                out=at, in_=d["Abp"][g, :, ch * 16:(ch + 1) * 16, :])
            for i in range(16):
                pr = ch * 16 + i
                for h in range(3):
                    cw = P if h < 2 else D * 17 - 2 * P
                    mm(paT[0:cw, 6 * pr + 2 * h:6 * pr + 2 * h + 2],
                       at[:, i, h * P:h * P + cw],
                       LN_[:, 2 * pr:2 * pr + 2], start=True, stop=True)
        aT_sb = scratch.tile([P, 6 * NPAIR], f32, tag="aT_sb", name="aT_sb")
        V.tensor_copy(
            out=aT_sb[:, 0:2 * P].rearrange("p (h t r) -> p h t r", h=2, r=2),
            in_=paT.rearrange("p (t hh r) -> p t hh r", hh=3, r=2)[:, :, 0:2, :]
            .rearrange("p t h r -> p h t r"))
        V.tensor_copy(
            out=aT_sb[0:16, 2 * P:2 * P + P].rearrange("p (t r) -> p t r", r=2),
            in_=paT[0:16, :].rearrange("p (t hh r) -> p t hh r", hh=3, r=2)
            [:, :, 2:3, :].rearrange("p t h r -> p t (h r)"))
        for h in range(3):
            cw = P if h < 2 else D * 17 - 2 * P
            ptb = ptp.tile([P, P], f32, tag="ptp", name="ptp")
            T.transpose(ptb[:, 0:cw], aT_sb[0:cw, h * P:h * P + P],
                        ident[0:cw, 0:cw])
            V.tensor_copy(out=Abk[:, h * P:h * P + cw], in_=ptb[:, 0:cw])

        # ---- pz_mu = z @ A + b ----
        AbkV = Abk.rearrange("p (i e) -> p i e", e=17)
        tpm = scratch.tile([P, D * D], f32, tag="tpm", name="tpm")
        V.tensor_tensor(out=tpm.rearrange("p (e i) -> p e i", e=D),
                        in0=AbkV[:, :, 0:16].rearrange("p i e -> p e i"),
                        in1=_bc1(zt, D, D), op=OP.mult)
        pzA = scratch.tile([P, D], f32, tag="pzA", name="pzA")
        V.tensor_reduce(out=pzA, in_=tpm.rearrange("p (e i) -> p e i", e=D),
                        axis=AX.X, op=OP.add)
        pzmu = wt([P, D], "pzmu")
        V.tensor_tensor(out=pzmu, in0=pzA, in1=_diag(Abk, D, 17, off=16),
                        op=OP.add)
        st["pzmu"] = pzmu

        # ---- gt = [outer(kn, z) | kn] @ wra ----
        Y = scratch.tile([P, K * D + K], f32, tag="Y", name="Y")
        V.tensor_tensor(out=Y[:, 0:K * D].rearrange("p (k i) -> p k i", k=K),
                        in0=_bci(kn, K, D), in1=_bc1(zt, K, D), op=OP.mult)
        GP.tensor_copy(out=Y[:, K * D:], in_=kn)
        yts = []
        for c, (w_, cl) in enumerate(wra):
            pt = ptp.tile([P, P], f32, tag="ptp", name="ptp")
            T.transpose(pt[:cl, :], Y[:, c * P:c * P + cl], ident)
            yT = scratch.tile([P, P], f32, tag=f"yT{c}", name=f"yT{c}")
            nc.scalar.copy(out=yT[:cl, :], in_=pt[:cl, :])
            yts.append(yT)
        pg = pmm.tile([P, 512], f32, tag="pmm", name="pmm")
        for c, (w_, cl) in enumerate(wra):
            mm(pg[:, 0:X], yts[c][:cl, :], w_[:cl, 0:X],
               start=(c == 0), stop=(c == len(wra) - 1))

        # ---- ez net ----
        ein = scratch.tile([P, 2 * X + K], f32, tag="ein", name="ein")
        nc.scalar.copy(out=ein[:, 0:X], in_=pg[:, 0:X])
        GP.tensor_copy(out=ein[:, X:X + K], in_=kn)
        GP.tensor_copy(out=ein[:, X + K:], in_=in0[:, D + K:])
        exh = scratch.tile([P, 2 * X + K], f32, tag="ez_xh", name="ez_xh")
        layernorm(ein, exh, "ezl0")
        eh0 = scratch.tile([P, H], f32, tag="ez_h0", name="ez_h0")
        dense(exh, ezw0, H, True, eh0, "ezd0")
        exh1 = scratch.tile([P, H], f32, tag="ez_xh1", name="ez_xh1")
        layernorm(eh0, exh1, "ezl1")
        eh1 = scratch.tile([P, H], f32, tag="ez_h1", name="ez_h1")
        dense(exh1, ezw1, H, True, eh1, "ezd1")
        exh2 = scratch.tile([P, H], f32, tag="ez_xh2", name="ez_xh2")
        layernorm(eh1, exh2, "ezl2")
        qzmu = wt([P, D], "qzmu")
        dense(exh2, ezw2, 2 * D, False, qzmu, "ezd2", psum_lo=0)
        st["qzmu"] = qzmu

        # ---- LDL^T of Qk:  Qk = Lu diag(D) Lu^T ----
        Lu = wt([P, D * D], "Lu")           # unit-lower, strict lower stored
        GP.memset(Lu, 0.0)
        LDt = wt([P, D * D], "LDt")         # LD[i,j] = Lu[i,j]*D_j; diag = D
        Dinv = wt([P, D], "Dinv")
        LuV = Lu.rearrange("p (i j) -> p i j", j=D)
        LDV = LDt.rearrange("p (i j) -> p i j", j=D)
        QkV = Qk.rearrange("p (i j) -> p i j", j=D)
        V.tensor_copy(out=LDV[:, :, 0:1], in_=QkV[:, :, 0:1])
        V.reciprocal(out=Dinv[:, 0:1], in_=LDt[:, 0:1])
        V.tensor_scalar(out=LuV[:, 1:, 0:1], in0=LDV[:, 1:, 0:1],
                        scalar1=Dinv[:, 0:1], scalar2=None, op0=OP.mult)
        for j in range(1, D):
            nr = D - j
            tmpd = scratch.tile([P, nr * j], f32, tag="ch_tmp", name="ch_tmp")
            V.tensor_tensor(out=tmpd.rearrange("p (i t) -> p i t", t=j),
                            in0=LuV[:, j:, 0:j],
                            in1=_bc1(LDt[:, j * D:j * D + j], nr, j),
                            op=OP.mult)
            sd = scratch.tile([P, nr], f32, tag="ch_sd", name="ch_sd")
            V.tensor_reduce(out=sd, in_=tmpd.rearrange("p (i t) -> p i t", t=j),
                            axis=AX.X, op=OP.add)
            V.tensor_tensor(out=LDV[:, j:, j:j + 1], in0=QkV[:, j:, j:j + 1],
                            in1=sd.rearrange("p (i o) -> p i o", o=1),
                            op=OP.subtract)
            V.reciprocal(out=Dinv[:, j:j + 1],
                         in_=LDt[:, j * D + j:j * D + j + 1])
            if j < D - 1:
                V.tensor_scalar(out=LuV[:, j + 1:, j:j + 1],
                                in0=LDV[:, j + 1:, j:j + 1],
                                scalar1=Dinv[:, j:j + 1], scalar2=None,
                                op0=OP.mult)
        st["LDt"], st["Dinv"] = LDt, Dinv

        Dc = scratch.tile([P, D], f32, tag="Dc", name="Dc")
        V.tensor_copy(out=Dc, in_=_diag(LDt, D, D + 1))
        rD = scratch.tile([P, D], f32, tag="rD", name="rD")
        newton_rsqrt(rD, Dc, "rD")
        sqD = scratch.tile([P, D], f32, tag="sqD", name="sqD")
        V.tensor_tensor(out=sqD, in0=Dc, in1=rD, op=OP.mult)

        # ---- sample ----
        u = scratch.tile([P, D], f32, tag="u_t", name="u_t")
        V.tensor_tensor(out=u, in0=sqD, in1=st["ept"], op=OP.mult)
        tl = scratch.tile([P, D * D], f32, tag="tl", name="tl")
        V.tensor_tensor(out=tl.rearrange("p (i t) -> p i t", t=D),
                        in0=LuV, in1=_bc1(u, D, D), op=OP.mult)
        Lu0 = scratch.tile([P, D], f32, tag="Lu0", name="Lu0")
        V.tensor_reduce(out=Lu0, in_=tl.rearrange("p (i t) -> p i t", t=D),
                        axis=AX.X, op=OP.add)
        Leps = scratch.tile([P, D], f32, tag="Leps", name="Leps")
        V.tensor_tensor(out=Leps, in0=Lu0, in1=u, op=OP.add)
        pzs = scratch.tile([P, D], f32, tag="pzs", name="pzs")
        V.tensor_tensor(out=pzs, in0=Leps, in1=pzmu, op=OP.add)
        V.tensor_scalar(out=pzs, in0=pzs, scalar1=100.0, scalar2=-100.0,
                        op0=OP.min, op1=OP.max)
        qse = scratch.tile([P, D], f32, tag="qse", name="qse")
        V.tensor_tensor(out=qse, in0=qsig, in1=st["ept"], op=OP.mult)
        V.tensor_tensor(out=qse, in0=qse, in1=st["qzmu"], op=OP.add)
        samp = wt([P, D], "samp")
        GP.tensor_copy(out=samp, in_=pzs)
        V.copy_predicated(out=samp, mask=st["mfk"][:, 0:1].broadcast_to((P, D)),
                          data=qse)
        st["samp"] = samp

        # ---- B build + unit-lower forward substitution (in place) ----
        B = wt([P, D * NB], "Bx")
        GP.memset(B, 0.0)
        GP.tensor_copy(out=_diag(B, D, NB + 1), in_=qsig)
        BV = B.rearrange("p (i c) -> p i c", c=NB)
        V.tensor_tensor(out=BV[:, :, D:D + 1],
                        in0=st["qzmu"].rearrange("p (i o) -> p i o", o=1),
                        in1=pzmu.rearrange("p (i o) -> p i o", o=1),
                        op=OP.subtract)
        V.tensor_tensor(out=BV[:, :, D + 1:D + 2],
                        in0=samp.rearrange("p (i o) -> p i o", o=1),
                        in1=pzmu.rearrange("p (i o) -> p i o", o=1),
                        op=OP.subtract)
        BP = B.rearrange("p (t c) -> p c t", c=NB)      # [P, NB, D]
        for i in range(1, D):
            tms = scratch.tile([P, NB * i], f32, tag="sb_tm", name="sb_tm")
            V.tensor_tensor(out=tms.rearrange("p (c t) -> p c t", t=i),
                            in0=BP[:, :, 0:i],
                            in1=_bc1(Lu[:, i * D:i * D + i], NB, i),
                            op=OP.mult)
            sv = scratch.tile([P, NB], f32, tag="sb_sv", name="sb_sv")
            V.tensor_reduce(out=sv, in_=tms.rearrange("p (c t) -> p c t", t=i),
                            axis=AX.X, op=OP.add)
            V.tensor_tensor(out=B[:, i * NB:(i + 1) * NB],
                            in0=B[:, i * NB:(i + 1) * NB], in1=sv,
                            op=OP.subtract)

        # ---- weighted norms:  sum_i row_i^2 * Dinv_i ----
        sqX = scratch.tile([P, D * NB], f32, tag="sqX", name="sqX")
        V.tensor_tensor(out=sqX, in0=B, in1=B, op=OP.mult)
        V.tensor_tensor(out=sqX.rearrange("p (i c) -> p i c", c=NB),
                        in0=sqX.rearrange("p (i c) -> p i c", c=NB),
                        in1=_bci(Dinv, D, NB), op=OP.mult)
        sqP = sqX.rearrange("p (i c) -> p c i", c=NB)    # [P, NB, D]
        tF = wt([P, 1], "tF")
        V.tensor_reduce(out=tF,
                        in_=sqX.rearrange("p (i c) -> p i c", c=NB)[:, :, 0:D],
                        axis=AX.XY, op=OP.add)
        tY = wt([P, 1], "tY")
        V.tensor_reduce(out=tY, in_=sqP[:, D:D + 1, :], axis=AX.X, op=OP.add)
        tW = wt([P, 1], "tW")
        V.tensor_reduce(out=tW, in_=sqP[:, D + 1:D + 2, :], axis=AX.X, op=OP.add)
        st["tF"], st["tY"], st["tW"] = tF, tY, tW

    # ============ PHASE D: ln/exp set — logdet, lps, kl, final softmax ======
    for g in range(NG):
        st = G[g]
        r0 = g * P
        osc = wt([P, 4], "osc")
        GP.memset(osc, 0.0)

        jk16 = scratch.tile([P, D], f32, tag="jk16", name="jk16")
        sld = scratch.tile([P, 1], f32, tag="sld", name="sld")       # logdet_p = sum ln D
        S.activation(out=jk16, in_=_diag(st["LDt"], D, D + 1),
                     func=AF.Ln, accum_out=sld)

        klA = scratch.tile([P, 1], f32, tag="klA", name="klA")
        V.tensor_tensor(out=klA, in0=st["tF"], in1=st["tY"], op=OP.add)
        klB = scratch.tile([P, 1], f32, tag="klB", name="klB")
        V.tensor_scalar(out=klB, in0=klA, scalar1=0.5, scalar2=-0.5 * D,
                        op0=OP.mult, op1=OP.add)
        klC = scratch.tile([P, 1], f32, tag="klC", name="klC")
        STT(out=klC, in0=sld, scalar=0.5, in1=klB, op0=OP.mult, op1=OP.add)
        klf = scratch.tile([P, 1], f32, tag="klf", name="klf")
        V.tensor_tensor(out=klf, in0=klC, in1=slsig, op=OP.subtract)
        V.tensor_tensor(out=osc[:, 0:1], in0=klf, in1=st["diw"], op=OP.add)
        GP.tensor_copy(out=osc[:, 1:2], in_=st["diw"])

        dq = scratch.tile([P, D], f32, tag="dq", name="dq")
        V.tensor_tensor(out=dq, in0=st["samp"], in1=st["qzmu"], op=OP.subtract)
        V.tensor_tensor(out=dq, in0=dq, in1=rsig, op=OP.mult)
        jkq = scratch.tile([P, D], f32, tag="jkq", name="jkq")
        sq2 = scratch.tile([P, 1], f32, tag="sq2", name="sq2")
        S.activation(out=jkq, in_=dq, func=AF.Square, accum_out=sq2)
        ql = scratch.tile([P, 1], f32, tag="ql", name="ql")
        V.tensor_scalar(out=ql, in0=sq2, scalar1=-0.5,
                        scalar2=-0.5 * D * LOG2PI, op0=OP.mult, op1=OP.add)
        V.tensor_tensor(out=ql, in0=ql, in1=slsig, op=OP.subtract)
        pl = scratch.tile([P, 1], f32, tag="pl", name="pl")
        V.tensor_scalar(out=pl, in0=st["tW"], scalar1=-0.5,
                        scalar2=-0.5 * D * LOG2PI, op0=OP.mult, op1=OP.add)
        STT(out=pl, in0=sld, scalar=-0.5, in1=pl, op0=OP.mult, op1=OP.add)
        V.tensor_tensor(out=osc[:, 2:3], in0=ql, in1=pl, op=OP.subtract)

        nm = scratch.tile([P, 1], f32, tag="fs_nm", name="fs_nm")
        V.tensor_reduce(out=nm, in_=st["qk"], axis=AX.X, op=OP.max, negate=True)
        ex = scratch.tile([P, K], f32, tag="fs_ex", name="fs_ex")
        ssum = scratch.tile([P, 1], f32, tag="fs_ss", name="fs_ss")
        S.activation(out=ex, in_=st["qk"], func=AF.Exp, bias=nm, accum_out=ssum)
        rs = scratch.tile([P, 1], f32, tag="fs_rs", name="fs_rs")
        V.reciprocal(out=rs, in_=ssum)
        oqk = wt([P, K], "oqk")
        V.tensor_scalar(out=oqk, in0=ex, scalar1=rs, scalar2=None, op0=OP.mult)

        nc.gpsimd.dma_start(out=d["o_sample"][r0:r0 + P, :], in_=st["samp"])
        nc.gpsimd.dma_start(out=d["o_sc"][r0:r0 + P, :], in_=osc)
        nc.gpsimd.dma_start(out=d["o_qk"][r0:r0 + P, :], in_=oqk)

    ctx.close()


# --------------------------------------------------------------------------
# host side
# --------------------------------------------------------------------------

def _gumbel_eps():
    import jax
    import jax.numpy as jnp
    cpu = jax.devices("cpu")[0]
    with jax.default_device(cpu):
        u = jax.random.uniform(jax.random.key(1), (N, K),
                               minval=1e-20, maxval=1.0)
        g = -jnp.log(-jnp.log(u))
        ep = jax.random.normal(jax.random.key(2), (N, D))
    return np.asarray(g, np.float32), np.asarray(ep, np.float32)


def make_in_maps(inputs):
    g_all, ep_all = _gumbel_eps()
    z = np.ascontiguousarray(np.asarray(inputs["z_sample"], np.float32))
    ks = np.ascontiguousarray(np.asarray(inputs["k_sample"], np.float32))
    xt = np.ascontiguousarray(np.asarray(inputs["xt"], np.float32))
    mask = np.asarray(inputs["mask"]).astype(np.int32).reshape(N, 1)
    def perm(x, f):
        # [NC(rows), 64, f] -> [NG, P=(two,k), NPAIR, f] per core shard
        x = x.reshape(NCORES, NG, NPAIR, 2, K, f)
        return np.ascontiguousarray(
            x.transpose(0, 1, 3, 4, 2, 5).reshape(NCORES, NG, P, NPAIR, f)
            .astype(np.float16))

    tr = perm(np.asarray(inputs["transition"], np.float32).reshape(N, K, K),
              K)
    Ab = perm(np.asarray(inputs["Ab"], np.float32).reshape(N, K, D * 17),
              D * 17)
    Q = perm(np.asarray(inputs["Q"], np.float32).reshape(N, K, D * D),
             D * D)
    W = np.asarray(inputs["W"], np.float32)
    c = np.asarray(inputs["c"], np.float32)
    wra = np.ascontiguousarray(np.concatenate([W.reshape(K * D, X), c], axis=0))
    qzs = np.ascontiguousarray(np.asarray(inputs["qz_sigma"], np.float32))
    tempv = np.array([[np.float32(inputs["temp"])]], np.float32)

    rep = {
        "fzw0": np.ascontiguousarray(np.asarray(inputs["fz_w0"], np.float32)),
        "fzw1": np.ascontiguousarray(np.asarray(inputs["fz_w1"], np.float32)),
        "fzw2": np.ascontiguousarray(np.asarray(inputs["fz_w2"], np.float32)),
        "ezw0": np.ascontiguousarray(np.asarray(inputs["ez_w0"], np.float32)),
        "ezw1": np.ascontiguousarray(np.asarray(inputs["ez_w1"], np.float32)),
        "ezw2": np.ascontiguousarray(np.asarray(inputs["ez_w2"], np.float32)),
        "wra": wra, "qzs": qzs, "tempv": tempv,
    }
    maps = []
    for ci in range(NCORES):
        lo, hi = ci * NC, (ci + 1) * NC
        m = {
            "z": z[lo:hi], "ks": ks[lo:hi], "xt": xt[lo:hi],
            "maskf": mask[lo:hi], "gn": g_all[lo:hi], "eps": ep_all[lo:hi],
            "trans": tr[ci], "Qp": Q[ci], "Abp": Ab[ci],
        }
        m.update(rep)
        maps.append(m)
    return maps


def assemble(results):
    sample = np.concatenate([np.asarray(r["o_sample"]) for r in results], axis=0)
    sc = np.concatenate([np.asarray(r["o_sc"]) for r in results], axis=0)
    oqk = np.concatenate([np.asarray(r["o_qk"]) for r in results], axis=0)
    out2 = sc[:, 0]
    diw = sc[:, 1]
    qpz = sc[:, 2]
    iwae = np.float32(qpz.astype(np.float32).sum())
    out3 = iwae + diw
    return (np.ascontiguousarray(sample, np.float32),
            np.ascontiguousarray(out2, np.float32),
            np.ascontiguousarray(out3, np.float32),
            np.ascontiguousarray(oqk, np.float32))


def kernel(**inputs):
    global _PROGRAM
    if _PROGRAM is None:
        _PROGRAM = build_program()
    nc, _ = _PROGRAM
    from concourse.bass_utils import run_bass_kernel_spmd
    maps = make_in_maps(inputs)
    res = run_bass_kernel_spmd(nc, maps, list(range(NCORES)))
    return assemble(res.results)


# revision 25
# speedup vs baseline: 1.0322x; 1.0322x over previous
"""Trainium2 Bass kernel for nn_DSLDSCell (moe_routing).

Data-parallel over 8 NeuronCores: each core processes 512 of the 4096 rows.
Heavy per-row k-weighted reductions (transition/Q/Ab contra k_new) run on the
TensorEngine as 2-rows-per-matmul block-diagonal contractions (k=64 on
partitions; 2 consecutive rows fill 128 partitions and are DRAM-contiguous).
Per-row 16x16 factorization uses LDL^T (no sqrt in the recurrence),
vectorized across 128 partitions. LayerNorm rsqrt is DVE-Newton (bit-trick
seed), keeping the ACT engine on only two table sets (gelu / ln+exp).
"""
import math
import sys

for _p in ("/opt/trn_rl_repo",):
    if _p not in sys.path:
        sys.path.insert(0, _p)

import numpy as np

import concourse.bass as bass
import concourse.bacc as bacc
import concourse.tile as tile
from concourse import mybir
from concourse.masks import make_identity

f32 = mybir.dt.float32
i32 = mybir.dt.int32
OP = mybir.AluOpType
AF = mybir.ActivationFunctionType
AX = mybir.AxisListType

N, D, K, X, H = 4096, 16, 64, 128, 256
NCORES = 8
NC = N // NCORES          # rows per core (512)
P = 128                   # partitions
NG = NC // P              # groups per core (4)
NPAIR = P // 2            # pairs per group (64)
NB = D + 2                # rhs columns in the triangular solve
LOG2PI = math.log(2.0 * math.pi)
MAGIC = 0x5F3759DF

f16 = mybir.dt.float16
GELU = "act"                # "act" = ACT Gelu_apprx_tanh; "tanh" = composite
                            # (CoreSim does not implement Gelu)

_PROGRAM = None


# --------------------------------------------------------------------------
# device program
# --------------------------------------------------------------------------

def _patch_ldw_opt():
    """The default walrus invocation passes --enable-ldw-opt=false; our
    stream matmuls are LDWEIGHTS-bound (fp16, 128-column weights), so
    re-enable the fast-weight-load path for this kernel's compiles."""
    import concourse.bass_utils as _bu
    if getattr(_bu, "_ldw_patched", False):
        return
    _orig = _bu.run_command

    def patched(cmd, **kw):
        cmd = ["--enable-ldw-opt=true" if c == "--enable-ldw-opt=false" else c
               for c in cmd]
        return _orig(cmd, **kw)

    _bu.run_command = patched
    _bu._ldw_patched = True


def _patch_act_tables():
    """Force walrus's table-load pass to place Exp/Ln/Square in
    natural_log_exp_and_others and Gelu_apprx_tanh in its own set, so the
    kernel only ever loads two ACT table sets (indices stay file-aligned)."""
    import concourse.bacc as _bacc
    if getattr(_bacc, "_act_tables_patched", False):
        return
    _orig = _bacc.get_activation_tables

    def patched(arch):
        t = _orig(arch)
        keep = {"natural_log_exp_and_others", "gelu_apprx_tanh_and_others"}
        drop = {AF.Exp, AF.Ln, AF.Square, AF.Gelu_apprx_tanh, AF.Tanh}
        for name, fns in t.items():
            if name not in keep:
                for f in drop:
                    fns.discard(f)
        return t

    _bacc.get_activation_tables = patched
    _bacc._act_tables_patched = True


def build_program():
    _patch_act_tables()
    nc = bacc.Bacc("TRN2", debug=False, num_devices=NCORES)

    dd = {}
    def din(name, shape, dt=f32):
        dd[name] = nc.declare_dram_parameter(name, shape, dt, isOutput=False)
    def dout(name, shape):
        dd[name] = nc.declare_dram_parameter(name, shape, f32, isOutput=True)

    din("z", [NC, D]); din("ks", [NC, K]); din("xt", [NC, X])
    din("maskf", [NC, 1], i32); din("gn", [NC, K]); din("eps", [NC, D])
    din("trans", [NG, P, NPAIR, K], f16)
    din("Qp", [NG, P, NPAIR, D * D], f16)
    din("Abp", [NG, P, NPAIR, D * 17], f16)
    din("fzw0", [D + K + X, H]); din("fzw1", [H, H]); din("fzw2", [H, K])
    din("ezw0", [2 * X + K, H]); din("ezw1", [H, H]); din("ezw2", [H, 2 * D])
    din("wra", [K * D + K, X]); din("qzs", [1, D]); din("tempv", [1, 1])
    dout("o_sample", [NC, D]); dout("o_sc", [NC, 4]); dout("o_qk", [NC, K])

    with tile.TileContext(nc) as tc:
        _emit(nc, tc, dd)
    nc.compile()
    return nc, ["o_sample", "o_sc", "o_qk"]


def _bc1(ap, mid, inner):
    """[P, inner] AP -> [P, mid, inner] with zero-stride middle dim."""
    return ap.rearrange("p (a n) -> p a n", a=1).broadcast_to(
        (ap.shape[0], mid, inner))


def _bci(ap, nk, inner):
    """[P, nk] AP -> [P, nk, inner] with zero-stride inner dim."""
    return ap.rearrange("p (k o) -> p k o", o=1).broadcast_to(
        (ap.shape[0], nk, inner))


def _diag(t, n, stride, off=0):
    """[P, n] view of diagonal-ish elements: offset + i*stride."""
    return bass.AP(tensor=t.tensor, offset=t.offset + off,
                   ap=[t.ap[0], [stride, n]])


def _emit(nc, tc, d):
    import contextlib
    ctx = contextlib.ExitStack()
    consts = ctx.enter_context(tc.tile_pool(name="consts", bufs=1))
    work = ctx.enter_context(tc.tile_pool(name="work", bufs=NG + 1))
    scratch = ctx.enter_context(tc.tile_pool(name="scratch", bufs=2))
    stream = ctx.enter_context(tc.tile_pool(name="stream", bufs=2))
    pmm = ctx.enter_context(tc.tile_pool(name="pmm", bufs=2, space="PSUM"))
    ptp = ctx.enter_context(tc.tile_pool(name="ptp", bufs=2, space="PSUM"))
    pstr = ctx.enter_context(tc.tile_pool(name="pstr", bufs=1, space="PSUM"))

    V, T = nc.vector, nc.tensor
    GP = nc.gpsimd
    STT = nc.vector.scalar_tensor_tensor
    _last_act = [None]

    class _SWrap:
        """Chain ACT ops in emission order so the scheduler cannot
        interleave activation-table sets across phases."""
        @staticmethod
        def activation(*a, **kw):
            bi = nc.scalar.activation(*a, **kw)
            if _last_act[0] is not None:
                bass._add_dep_helper(bi.ins, _last_act[0], sync=False,
                                     reason="act-table-order")
            _last_act[0] = bi.ins
            return bi
    S = _SWrap

    # ---------------- constants ----------------
    ident = consts.tile([P, P], f32)
    make_identity(nc, ident)
    magic16 = consts.tile([P, 16], i32)
    V.memset(magic16, MAGIC)
    zero_c = consts.tile([P, P], f32)
    V.memset(zero_c, 0.0)

    def newton_rsqrt(dst, y, tagp, iters=3):
        """dst = 1/sqrt(y); y, dst [P, w] f32 contiguous, w <= 16."""
        w = y.shape[-1]
        hv = scratch.tile([P, w], i32, tag=tagp + "_h", name=tagp + "_h")
        V.tensor_scalar(out=hv, in0=y.bitcast(i32), scalar1=1, scalar2=None,
                        op0=OP.logical_shift_right)
        V.tensor_tensor(out=dst.bitcast(i32), in0=magic16[:, 0:w], in1=hv,
                        op=OP.subtract)
        t = scratch.tile([P, w], f32, tag=tagp + "_t", name=tagp + "_t")
        e = scratch.tile([P, w], f32, tag=tagp + "_e", name=tagp + "_e")
        for _ in range(iters):
            if w == 1:
                STT(out=t, in0=dst, scalar=dst, in1=y, op0=OP.mult, op1=OP.mult)
            else:
                V.tensor_tensor(out=t, in0=dst, in1=dst, op=OP.mult)
                V.tensor_tensor(out=t, in0=t, in1=y, op=OP.mult)
            V.tensor_scalar(out=e, in0=t, scalar1=-0.5, scalar2=1.5,
                            op0=OP.mult, op1=OP.add)
            V.tensor_tensor(out=dst, in0=dst, in1=e, op=OP.mult)

    tempb = consts.tile([P, 1], f32)
    nc.sync.dma_start(out=tempb, in_=bass.AP(
        tensor=d["tempv"], offset=0, ap=[[0, P], [1, 1]]))
    invt = consts.tile([P, 1], f32)
    V.reciprocal(out=invt, in_=tempb)
    ntmp = consts.tile([P, 1], f32)
    V.tensor_scalar(out=ntmp, in0=tempb, scalar1=-1.0, scalar2=None,
                    op0=OP.mult)

    qzs_b = consts.tile([P, D], f32)
    nc.sync.dma_start(out=qzs_b, in_=bass.AP(
        tensor=d["qzs"], offset=0, ap=[[0, P], [1, D]]))
    qzs_cl = consts.tile([P, D], f32)
    V.tensor_scalar(out=qzs_cl, in0=qzs_b, scalar1=-3.0, scalar2=None,
                    op0=OP.max)
    qsig = consts.tile([P, D], f32)
    S.activation(out=qsig, in_=qzs_cl, func=AF.Exp)
    rsig = consts.tile([P, D], f32)
    V.reciprocal(out=rsig, in_=qsig)
    slsig = consts.tile([P, 1], f32)
    V.tensor_reduce(out=slsig, in_=qzs_cl, axis=AX.X, op=OP.add)

    def load_w(name, dn, do):
        chunks = []
        for c in range((dn + P - 1) // P):
            cl = min(P, dn - c * P)
            t = consts.tile([P, do], f32, tag=f"{name}{c}", name=f"{name}{c}")
            nc.sync.dma_start(out=t[:cl, :], in_=d[name][c * P:c * P + cl, :])
            chunks.append((t, cl))
        return chunks

    fzw0 = load_w("fzw0", D + K + X, H)
    fzw1 = load_w("fzw1", H, H)
    fzw2 = load_w("fzw2", H, K)
    ezw0 = load_w("ezw0", 2 * X + K, H)
    ezw1 = load_w("ezw1", H, H)
    ezw2 = load_w("ezw2", H, 2 * D)
    wra = load_w("wra", K * D + K, X)

    def mm(out, lhsT, rhs, **kw):
        T.matmul(out, lhsT, rhs, **kw)

    def layernorm(x, out, tagp):
        st = scratch.tile([P, 6], f32, tag=tagp + "_st", name=tagp + "_st")
        V.bn_stats(out=st, in_=x)
        mv = scratch.tile([P, 2], f32, tag=tagp + "_mv", name=tagp + "_mv")
        V.bn_aggr(out=mv, in_=st)
        ve = scratch.tile([P, 1], f32, tag=tagp + "_ve", name=tagp + "_ve")
        V.tensor_scalar(out=ve, in0=mv[:, 1:2], scalar1=1e-6, scalar2=None,
                        op0=OP.add)
        rst = scratch.tile([P, 1], f32, tag=tagp + "_rs", name=tagp + "_rs")
        newton_rsqrt(rst, ve, tagp)
        V.tensor_scalar(out=out, in0=x, scalar1=mv[:, 0:1], scalar2=rst,
                        op0=OP.subtract, op1=OP.mult)

    def dense(x, wchunks, do, gelu, out_sb, tagp, psum_lo=0):
        """out_sb = [gelu](x @ W)[:, psum_lo:psum_lo+width(out_sb)]."""
        xts = []
        for c, (w_, cl) in enumerate(wchunks):
            pt = ptp.tile([P, P], f32, tag="ptp", name="ptp")
            T.transpose(pt[:cl, :], x[:, c * P:c * P + cl], ident)
            xT = scratch.tile([P, P], f32, tag=tagp + f"_xT{c}", name=tagp + f"_xT{c}")
            nc.scalar.copy(out=xT[:cl, :], in_=pt[:cl, :])
            xts.append(xT)
        ph = pmm.tile([P, 512], f32, tag="pmm", name="pmm")
        nchunk = len(wchunks)
        for c, (w_, cl) in enumerate(wchunks):
            mm(ph[:, 0:do], xts[c][:cl, :], w_[:cl, 0:do],
               start=(c == 0), stop=(c == nchunk - 1))
        wo = out_sb.shape[-1]
        if gelu and GELU == "act":
            S.activation(out=out_sb, in_=ph[:, psum_lo:psum_lo + wo],
                         func=AF.Gelu_apprx_tanh)
        elif gelu:
            xs = scratch.tile([P, wo], f32, tag=tagp + "_gx", name=tagp + "_gx")
            V.tensor_copy(out=xs, in_=ph[:, psum_lo:psum_lo + wo])
            t3 = scratch.tile([P, wo], f32, tag=tagp + "_g3", name=tagp + "_g3")
            V.tensor_tensor(out=t3, in0=xs, in1=xs, op=OP.mult)
            V.tensor_tensor(out=t3, in0=t3, in1=xs, op=OP.mult)
            STT(out=t3, in0=t3, scalar=0.044715, in1=xs, op0=OP.mult, op1=OP.add)
            S.activation(out=t3, in_=t3, func=AF.Tanh,
                         scale=0.7978845608028654)
            V.tensor_scalar(out=t3, in0=t3, scalar1=0.5, scalar2=0.5,
                            op0=OP.mult, op1=OP.add)
            V.tensor_tensor(out=out_sb, in0=t3, in1=xs, op=OP.mult)
        else:
            V.tensor_copy(out=out_sb, in_=ph[:, psum_lo:psum_lo + wo])

    def blockdiag_lhsT(src64, tag):
        """src64: [P, K] rows tile -> transpose -> [128,128] block-diag
        columns: col 2t+0 = src row 2t on parts 0:64, col 2t+1 = row 2t+1
        on parts 64:128."""
        pt = ptp.tile([P, P], f32, tag="ptp", name="ptp")
        T.transpose(pt[0:K, :], src64, ident)
        sT = scratch.tile([K, P], f16, tag=tag + "_T", name=tag + "_T")
        nc.scalar.copy(out=sT, in_=pt[0:K, :])
        LA = scratch.tile([P, P], f16, tag=tag + "_LA", name=tag + "_LA")
        GP.memset(LA, 0.0)
        GP.tensor_copy(
            out=LA[0:K, :].rearrange("p (t two) -> p t two", two=2)[:, :, 0:1],
            in_=sT.rearrange("p (t two) -> p t two", two=2)[:, :, 0:1])
        GP.tensor_copy(
            out=LA[K:P, :].rearrange("p (t two) -> p t two", two=2)[:, :, 1:2],
            in_=sT.rearrange("p (t two) -> p t two", two=2)[:, :, 1:2])
        return LA

    G = {}

    def wt(shape, tag):
        return work.tile(shape, f32, tag=tag, name=tag)

    # ============ PHASE A: inputs, fz net (gelu set), transition MMs ========
    for g in range(NG):
        r0 = g * P
        st = {}
        G[g] = st
        in0 = wt([P, D + K + X], "in0")
        nc.sync.dma_start(out=in0[:, 0:D], in_=d["z"][r0:r0 + P, :])
        nc.sync.dma_start(out=in0[:, D:D + K], in_=d["ks"][r0:r0 + P, :])
        nc.sync.dma_start(out=in0[:, D + K:], in_=d["xt"][r0:r0 + P, :])
        st["in0"] = in0
        mfk = work.tile([P, 1], i32, tag="mfk", name="mfk")
        nc.sync.dma_start(out=mfk, in_=d["maskf"][r0:r0 + P, :])
        st["mfk"] = mfk
        gnt = wt([P, K], "gnt")
        nc.sync.dma_start(out=gnt, in_=d["gn"][r0:r0 + P, :])
        st["gnt"] = gnt
        ept = wt([P, D], "ept")
        nc.sync.dma_start(out=ept, in_=d["eps"][r0:r0 + P, :])
        st["ept"] = ept

        # ---- fz net ----
        xh = scratch.tile([P, D + K + X], f32, tag="fz_xh", name="fz_xh")
        layernorm(in0, xh, "fzl0")
        h0 = scratch.tile([P, H], f32, tag="fz_h0", name="fz_h0")
        dense(xh, fzw0, H, True, h0, "fzd0")
        xh1 = scratch.tile([P, H], f32, tag="fz_xh1", name="fz_xh1")
        layernorm(h0, xh1, "fzl1")
        h1 = scratch.tile([P, H], f32, tag="fz_h1", name="fz_h1")
        dense(xh1, fzw1, H, True, h1, "fzd1")
        xh2 = scratch.tile([P, H], f32, tag="fz_xh2", name="fz_xh2")
        layernorm(h1, xh2, "fzl2")
        qk = wt([P, K], "qk")
        dense(xh2, fzw2, K, False, qk, "fzd2")
        st["qk"] = qk

        # ---- transition pair-MMs (data stationary, kn-blockdiag moving) ----
        # out[j, r] per pair -> psum_tT [64, 2t+r]; transpose back afterwards.
        LK = blockdiag_lhsT(in0[:, D:D + K], "kf")
        pairbase = g * NPAIR
        ptT = pstr.tile([K, P], f32, tag="ptT", name="ptT")
        for ch in range(4):                       # 4 chunks of 16 pairs
            trt = stream.tile([P, 16, K], f16, tag="trch", name="trch")
            nc.sync.dma_start(
                out=trt, in_=d["trans"][g, :, ch * 16:(ch + 1) * 16, :])
            for i in range(16):
                pr = ch * 16 + i
                mm(ptT[:, 2 * pr:2 * pr + 2], trt[:, i, :],
                   LK[:, 2 * pr:2 * pr + 2], start=True, stop=True)
        tT_sb = scratch.tile([K, P], f32, tag="tT_sb", name="tT_sb")
        nc.scalar.copy(out=tT_sb, in_=ptT)
        ptb = ptp.tile([P, P], f32, tag="ptp", name="ptp")
        T.transpose(ptb[:, 0:K], tT_sb, ident[0:K, 0:K])
        pkp = wt([P, K], "pkp")
        V.tensor_copy(out=pkp, in_=ptb[:, 0:K])
        st["pkp"] = pkp

    # ============ PHASE B: ln/exp set — pk_logits, softmaxes, k_new, d_iwae =
    for g in range(NG):
        st = G[g]
        pkl = scratch.tile([P, K], f32, tag="pkl", name="pkl")
        S.activation(out=pkl, in_=st["pkp"], func=AF.Ln)

        gt_n = scratch.tile([P, K], f32, tag="gt_n", name="gt_n")
        V.tensor_scalar(out=gt_n, in0=st["gnt"], scalar1=invt[:, 0:1],
                        scalar2=None, op0=OP.mult)

        def softmax_t(logits, tagp, out_tile):
            sx = scratch.tile([P, K], f32, tag=tagp + "_sx", name=tagp + "_sx")
            STT(out=sx, in0=logits, scalar=invt[:, 0:1], in1=gt_n,
                op0=OP.mult, op1=OP.add)
            nm = scratch.tile([P, 1], f32, tag=tagp + "_nm", name=tagp + "_nm")
            V.tensor_reduce(out=nm, in_=sx, axis=AX.X, op=OP.max, negate=True)
            ex = scratch.tile([P, K], f32, tag=tagp + "_ex", name=tagp + "_ex")
            ssum = scratch.tile([P, 1], f32, tag=tagp + "_ss", name=tagp + "_ss")
            S.activation(out=ex, in_=sx, func=AF.Exp, bias=nm, accum_out=ssum)
            rs = scratch.tile([P, 1], f32, tag=tagp + "_rs", name=tagp + "_rs")
            V.reciprocal(out=rs, in_=ssum)
            V.tensor_scalar(out=out_tile, in0=ex, scalar1=rs, scalar2=None,
                            op0=OP.mult)

        qks = scratch.tile([P, K], f32, tag="qks", name="qks")
        softmax_t(st["qk"], "smq", qks)
        pks = scratch.tile([P, K], f32, tag="pks", name="pks")
        softmax_t(pkl, "smp", pks)

        kn = wt([P, K], "kn")
        V.tensor_copy(out=kn, in_=pks)
        V.copy_predicated(out=kn, mask=st["mfk"][:, 0:1].broadcast_to((P, K)),
                          data=qks)
        st["kn"] = kn

        logx = scratch.tile([P, K], f32, tag="logx", name="logx")
        S.activation(out=logx, in_=kn, func=AF.Ln)

        def lse(logits, tagp, out_t):
            sc = scratch.tile([P, K], f32, tag=tagp + "_sc", name=tagp + "_sc")
            STT(out=sc, in0=logx, scalar=ntmp[:, 0:1], in1=logits,
                op0=OP.mult, op1=OP.add)
            nm = scratch.tile([P, 1], f32, tag=tagp + "_nm", name=tagp + "_nm")
            V.tensor_reduce(out=nm, in_=sc, axis=AX.X, op=OP.max, negate=True)
            ex = scratch.tile([P, K], f32, tag=tagp + "_ex", name=tagp + "_ex")
            sm = scratch.tile([P, 1], f32, tag=tagp + "_sm", name=tagp + "_sm")
            S.activation(out=ex, in_=sc, func=AF.Exp, bias=nm, accum_out=sm)
            ls = scratch.tile([P, 1], f32, tag=tagp + "_ls", name=tagp + "_ls")
            S.activation(out=ls, in_=sm, func=AF.Ln)
            V.tensor_tensor(out=out_t, in0=ls, in1=nm, op=OP.subtract)

        lq = scratch.tile([P, 1], f32, tag="lseq_o", name="lseq_o")
        lse(st["qk"], "lseq", lq)
        lp = scratch.tile([P, 1], f32, tag="lsep_o", name="lsep_o")
        lse(pkl, "lsep", lp)

        df = scratch.tile([P, K], f32, tag="df", name="df")
        V.tensor_tensor(out=df, in0=st["qk"], in1=pkl, op=OP.subtract)
        dsum = scratch.tile([P, 1], f32, tag="dsum", name="dsum")
        V.tensor_reduce(out=dsum, in_=df, axis=AX.X, op=OP.add)
        dl = scratch.tile([P, 1], f32, tag="dl", name="dl")
        V.tensor_tensor(out=dl, in0=lq, in1=lp, op=OP.subtract)
        diw = wt([P, 1], "diw")
        STT(out=diw, in0=dl, scalar=-float(K), in1=dsum,
            op0=OP.mult, op1=OP.add)
        st["diw"] = diw

    # ============ PHASE C: streams, gt, ez net (gelu), LDLT, solves =========
    for g in range(NG):
        st = G[g]
        in0 = st["in0"]
        kn = st["kn"]
        zt = in0[:, 0:D]
        pairbase = g * NPAIR

        LN_ = blockdiag_lhsT(kn, "kn")

        # ---- Q stream: out[de, r] per pair; psum_qT [128, 4t+2h+r] ----
        Qk = wt([P, D * D], "Qk")
        pqT = pstr.tile([P, 4 * NPAIR], f32, tag="pqT", name="pqT")
        for ch in range(4):
            qt = stream.tile([P, 16, D * D], f16, tag="qch", name="qch")
            nc.sync.dma_start(
                out=qt, in_=d["Qp"][g, :, ch * 16:(ch + 1) * 16, :])
            for i in range(16):
                pr = ch * 16 + i
                for h in range(2):
                    mm(pqT[:, 4 * pr + 2 * h:4 * pr + 2 * h + 2],
                       qt[:, i, h * P:(h + 1) * P],
                       LN_[:, 2 * pr:2 * pr + 2], start=True, stop=True)
        qT_sb = scratch.tile([P, 4 * NPAIR], f32, tag="qT_sb", name="qT_sb")
        V.tensor_copy(
            out=qT_sb.rearrange("p (h t r) -> p h t r", h=2, r=2),
            in_=pqT.rearrange("p (t h r) -> p h t r", h=2, r=2))
        for h in range(2):
            ptb = ptp.tile([P, P], f32, tag="ptp", name="ptp")
            T.transpose(ptb[:, :], qT_sb[:, h * P:(h + 1) * P], ident)
            V.tensor_copy(out=Qk[:, h * P:(h + 1) * P], in_=ptb[:, :])

        # ---- Ab stream: 3 de-chunks (128,128,16); psum_aT [128, 6t+2h+r] ----
        Abk = wt([P, D * 17], "Abk")
        paT = pstr.tile([P, 6 * NPAIR], f32, tag="paT", name="paT")
        for ch in range(4):
            at = stream.tile([P, 16, D * 17], f16, tag="abch", name="abch")
            nc.sync.dma_start(
                out=at, in_=d["Abp"][g, :, ch * 16:(ch + 1) * 16, :])
            for i in range(16):
                pr = ch * 16 + i
                for h in range(3):
                    cw = P if h < 2 else D * 17 - 2 * P
                    mm(paT[0:cw, 6 * pr + 2 * h:6 * pr + 2 * h + 2],
                       at[:, i, h * P:h * P + cw],
                       LN_[:, 2 * pr:2 * pr + 2], start=True, stop=True)
        aT_sb = scratch.tile([P, 6 * NPAIR], f32, tag="aT_sb", name="aT_sb")
        V.tensor_copy(
            out=aT_sb[:, 0:2 * P].rearrange("p (h t r) -> p h t r", h=2, r=2),
            in_=paT.rearrange("p (t hh r) -> p t hh r", hh=3, r=2)[:, :, 0:2, :]
            .rearrange("p t h r -> p h t r"))
        V.tensor_copy(
            out=aT_sb[0:16, 2 * P:2 * P + P].rearrange("p (t r) -> p t r", r=2),
            in_=paT[0:16, :].rearrange("p (t hh r) -> p t hh r", hh=3, r=2)
            [:, :, 2:3, :].rearrange("p t h r -> p t (h r)"))
        for h in range(3):
            cw = P if h < 2 else D * 17 - 2 * P
            ptb = ptp.tile([P, P], f32, tag="ptp", name="ptp")
            T.transpose(ptb[:, 0:cw], aT_sb[0:cw, h * P:h * P + P],
                        ident[0:cw, 0:cw])
            V.tensor_copy(out=Abk[:, h * P:h * P + cw], in_=ptb[:, 0:cw])

        # ---- pz_mu = z @ A + b ----
        AbkV = Abk.rearrange("p (i e) -> p i e", e=17)
        tpm = scratch.tile([P, D * D], f32, tag="tpm", name="tpm")
        V.tensor_tensor(out=tpm.rearrange("p (e i) -> p e i", e=D),
                        in0=AbkV[:, :, 0:16].rearrange("p i e -> p e i"),
                        in1=_bc1(zt, D, D), op=OP.mult)
        pzA = scratch.tile([P, D], f32, tag="pzA", name="pzA")
        V.tensor_reduce(out=pzA, in_=tpm.rearrange("p (e i) -> p e i", e=D),
                        axis=AX.X, op=OP.add)
        pzmu = wt([P, D], "pzmu")
        V.tensor_tensor(out=pzmu, in0=pzA, in1=_diag(Abk, D, 17, off=16),
                        op=OP.add)
        st["pzmu"] = pzmu

        # ---- gt = [outer(kn, z) | kn] @ wra ----
        Y = scratch.tile([P, K * D + K], f32, tag="Y", name="Y")
        V.tensor_tensor(out=Y[:, 0:K * D].rearrange("p (k i) -> p k i", k=K),
                        in0=_bci(kn, K, D), in1=_bc1(zt, K, D), op=OP.mult)
        GP.tensor_copy(out=Y[:, K * D:], in_=kn)
        yts = []
        for c, (w_, cl) in enumerate(wra):
            pt = ptp.tile([P, P], f32, tag="ptp", name="ptp")
            T.transpose(pt[:cl, :], Y[:, c * P:c * P + cl], ident)
            yT = scratch.tile([P, P], f32, tag=f"yT{c}", name=f"yT{c}")
            nc.scalar.copy(out=yT[:cl, :], in_=pt[:cl, :])
            yts.append(yT)
        pg = pmm.tile([P, 512], f32, tag="pmm", name="pmm")
        for c, (w_, cl) in enumerate(wra):
            mm(pg[:, 0:X], yts[c][:cl, :], w_[:cl, 0:X],
               start=(c == 0), stop=(c == len(wra) - 1))

        # ---- ez net ----
        ein = scratch.tile([P, 2 * X + K], f32, tag="ein", name="ein")
        nc.scalar.copy(out=ein[:, 0:X], in_=pg[:, 0:X])
        GP.tensor_copy(out=ein[:, X:X + K], in_=kn)
        GP.tensor_copy(out=ein[:, X + K:], in_=in0[:, D + K:])
        exh = scratch.tile([P, 2 * X + K], f32, tag="ez_xh", name="ez_xh")
        layernorm(ein, exh, "ezl0")
        eh0 = scratch.tile([P, H], f32, tag="ez_h0", name="ez_h0")
        dense(exh, ezw0, H, True, eh0, "ezd0")
        exh1 = scratch.tile([P, H], f32, tag="ez_xh1", name="ez_xh1")
        layernorm(eh0, exh1, "ezl1")
        eh1 = scratch.tile([P, H], f32, tag="ez_h1", name="ez_h1")
        dense(exh1, ezw1, H, True, eh1, "ezd1")
        exh2 = scratch.tile([P, H], f32, tag="ez_xh2", name="ez_xh2")
        layernorm(eh1, exh2, "ezl2")
        qzmu = wt([P, D], "qzmu")
        dense(exh2, ezw2, 2 * D, False, qzmu, "ezd2", psum_lo=0)
        st["qzmu"] = qzmu

        # ---- LDL^T of Qk:  Qk = Lu diag(D) Lu^T ----
        Lu = wt([P, D * D], "Lu")           # unit-lower, strict lower stored
        GP.memset(Lu, 0.0)
        LDt = wt([P, D * D], "LDt")         # LD[i,j] = Lu[i,j]*D_j; diag = D
        Dinv = wt([P, D], "Dinv")
        LuV = Lu.rearrange("p (i j) -> p i j", j=D)
        LDV = LDt.rearrange("p (i j) -> p i j", j=D)
        QkV = Qk.rearrange("p (i j) -> p i j", j=D)
        V.tensor_copy(out=LDV[:, :, 0:1], in_=QkV[:, :, 0:1])
        V.reciprocal(out=Dinv[:, 0:1], in_=LDt[:, 0:1])
        V.tensor_scalar(out=LuV[:, 1:, 0:1], in0=LDV[:, 1:, 0:1],
                        scalar1=Dinv[:, 0:1], scalar2=None, op0=OP.mult)
        for j in range(1, D):
            nr = D - j
            tmpd = scratch.tile([P, nr * j], f32, tag="ch_tmp", name="ch_tmp")
            V.tensor_tensor(out=tmpd.rearrange("p (i t) -> p i t", t=j),
                            in0=LuV[:, j:, 0:j],
                            in1=_bc1(LDt[:, j * D:j * D + j], nr, j),
                            op=OP.mult)
            sd = scratch.tile([P, nr], f32, tag="ch_sd", name="ch_sd")
            V.tensor_reduce(out=sd, in_=tmpd.rearrange("p (i t) -> p i t", t=j),
                            axis=AX.X, op=OP.add)
            V.tensor_tensor(out=LDV[:, j:, j:j + 1], in0=QkV[:, j:, j:j + 1],
                            in1=sd.rearrange("p (i o) -> p i o", o=1),
                            op=OP.subtract)
            V.reciprocal(out=Dinv[:, j:j + 1],
                         in_=LDt[:, j * D + j:j * D + j + 1])
            if j < D - 1:
                V.tensor_scalar(out=LuV[:, j + 1:, j:j + 1],
                                in0=LDV[:, j + 1:, j:j + 1],
                                scalar1=Dinv[:, j:j + 1], scalar2=None,
                                op0=OP.mult)
        st["LDt"], st["Dinv"] = LDt, Dinv

        Dc = scratch.tile([P, D], f32, tag="Dc", name="Dc")
        V.tensor_copy(out=Dc, in_=_diag(LDt, D, D + 1))
        rD = scratch.tile([P, D], f32, tag="rD", name="rD")
        newton_rsqrt(rD, Dc, "rD")
        sqD = scratch.tile([P, D], f32, tag="sqD", name="sqD")
        V.tensor_tensor(out=sqD, in0=Dc, in1=rD, op=OP.mult)

        # ---- sample ----
        u = scratch.tile([P, D], f32, tag="u_t", name="u_t")
        V.tensor_tensor(out=u, in0=sqD, in1=st["ept"], op=OP.mult)
        tl = scratch.tile([P, D * D], f32, tag="tl", name="tl")
        V.tensor_tensor(out=tl.rearrange("p (i t) -> p i t", t=D),
                        in0=LuV, in1=_bc1(u, D, D), op=OP.mult)
        Lu0 = scratch.tile([P, D], f32, tag="Lu0", name="Lu0")
        V.tensor_reduce(out=Lu0, in_=tl.rearrange("p (i t) -> p i t", t=D),
                        axis=AX.X, op=OP.add)
        Leps = scratch.tile([P, D], f32, tag="Leps", name="Leps")
        V.tensor_tensor(out=Leps, in0=Lu0, in1=u, op=OP.add)
        pzs = scratch.tile([P, D], f32, tag="pzs", name="pzs")
        V.tensor_tensor(out=pzs, in0=Leps, in1=pzmu, op=OP.add)
        V.tensor_scalar(out=pzs, in0=pzs, scalar1=100.0, scalar2=-100.0,
                        op0=OP.min, op1=OP.max)
        qse = scratch.tile([P, D], f32, tag="qse", name="qse")
        V.tensor_tensor(out=qse, in0=qsig, in1=st["ept"], op=OP.mult)
        V.tensor_tensor(out=qse, in0=qse, in1=st["qzmu"], op=OP.add)
        samp = wt([P, D], "samp")
        GP.tensor_copy(out=samp, in_=pzs)
        V.copy_predicated(out=samp, mask=st["mfk"][:, 0:1].broadcast_to((P, D)),
                          data=qse)
        st["samp"] = samp

        # ---- B build + unit-lower forward substitution (in place) ----
        B = wt([P, D * NB], "Bx")
        GP.memset(B, 0.0)
        GP.tensor_copy(out=_diag(B, D, NB + 1), in_=qsig)
        BV = B.rearrange("p (i c) -> p i c", c=NB)
        V.tensor_tensor(out=BV[:, :, D:D + 1],
                        in0=st["qzmu"].rearrange("p (i o) -> p i o", o=1),
                        in1=pzmu.rearrange("p (i o) -> p i o", o=1),
                        op=OP.subtract)
        V.tensor_tensor(out=BV[:, :, D + 1:D + 2],
                        in0=samp.rearrange("p (i o) -> p i o", o=1),
                        in1=pzmu.rearrange("p (i o) -> p i o", o=1),
                        op=OP.subtract)
        BP = B.rearrange("p (t c) -> p c t", c=NB)      # [P, NB, D]
        for i in range(1, D):
            tms = scratch.tile([P, NB * i], f32, tag="sb_tm", name="sb_tm")
            V.tensor_tensor(out=tms.rearrange("p (c t) -> p c t", t=i),
                            in0=BP[:, :, 0:i],
                            in1=_bc1(Lu[:, i * D:i * D + i], NB, i),
                            op=OP.mult)
            sv = scratch.tile([P, NB], f32, tag="sb_sv", name="sb_sv")
            V.tensor_reduce(out=sv, in_=tms.rearrange("p (c t) -> p c t", t=i),
                            axis=AX.X, op=OP.add)
            V.tensor_tensor(out=B[:, i * NB:(i + 1) * NB],
                            in0=B[:, i * NB:(i + 1) * NB], in1=sv,
                            op=OP.subtract)

        # ---- weighted norms:  sum_i row_i^2 * Dinv_i ----
        sqX = scratch.tile([P, D * NB], f32, tag="sqX", name="sqX")
        V.tensor_tensor(out=sqX, in0=B, in1=B, op=OP.mult)
        V.tensor_tensor(out=sqX.rearrange("p (i c) -> p i c", c=NB),
                        in0=sqX.rearrange("p (i c) -> p i c", c=NB),
                        in1=_bci(Dinv, D, NB), op=OP.mult)
        sqP = sqX.rearrange("p (i c) -> p c i", c=NB)    # [P, NB, D]
        tF = wt([P, 1], "tF")
        V.tensor_reduce(out=tF,
                        in_=sqX.rearrange("p (i c) -> p i c", c=NB)[:, :, 0:D],
                        axis=AX.XY, op=OP.add)
        tY = wt([P, 1], "tY")
        V.tensor_reduce(out=tY, in_=sqP[:, D:D + 1, :], axis=AX.X, op=OP.add)
        tW = wt([P, 1], "tW")
        V.tensor_reduce(out=tW, in_=sqP[:, D + 1:D + 2, :], axis=AX.X, op=OP.add)
        st["tF"], st["tY"], st["tW"] = tF, tY, tW

    # ============ PHASE D: ln/exp set — logdet, lps, kl, final softmax ======
    for g in range(NG):
        st = G[g]
        r0 = g * P
        osc = wt([P, 4], "osc")
        GP.memset(osc, 0.0)

        jk16 = scratch.tile([P, D], f32, tag="jk16", name="jk16")
        sld = scratch.tile([P, 1], f32, tag="sld", name="sld")       # logdet_p = sum ln D
        S.activation(out=jk16, in_=_diag(st["LDt"], D, D + 1),
                     func=AF.Ln, accum_out=sld)

        klA = scratch.tile([P, 1], f32, tag="klA", name="klA")
        V.tensor_tensor(out=klA, in0=st["tF"], in1=st["tY"], op=OP.add)
        klB = scratch.tile([P, 1], f32, tag="klB", name="klB")
        V.tensor_scalar(out=klB, in0=klA, scalar1=0.5, scalar2=-0.5 * D,
                        op0=OP.mult, op1=OP.add)
        klC = scratch.tile([P, 1], f32, tag="klC", name="klC")
        STT(out=klC, in0=sld, scalar=0.5, in1=klB, op0=OP.mult, op1=OP.add)
        klf = scratch.tile([P, 1], f32, tag="klf", name="klf")
        V.tensor_tensor(out=klf, in0=klC, in1=slsig, op=OP.subtract)
        V.tensor_tensor(out=osc[:, 0:1], in0=klf, in1=st["diw"], op=OP.add)
        GP.tensor_copy(out=osc[:, 1:2], in_=st["diw"])

        dq = scratch.tile([P, D], f32, tag="dq", name="dq")
        V.tensor_tensor(out=dq, in0=st["samp"], in1=st["qzmu"], op=OP.subtract)
        V.tensor_tensor(out=dq, in0=dq, in1=rsig, op=OP.mult)
        jkq = scratch.tile([P, D], f32, tag="jkq", name="jkq")
        sq2 = scratch.tile([P, 1], f32, tag="sq2", name="sq2")
        S.activation(out=jkq, in_=dq, func=AF.Square, accum_out=sq2)
        ql = scratch.tile([P, 1], f32, tag="ql", name="ql")
        V.tensor_scalar(out=ql, in0=sq2, scalar1=-0.5,
                        scalar2=-0.5 * D * LOG2PI, op0=OP.mult, op1=OP.add)
        V.tensor_tensor(out=ql, in0=ql, in1=slsig, op=OP.subtract)
        pl = scratch.tile([P, 1], f32, tag="pl", name="pl")
        V.tensor_scalar(out=pl, in0=st["tW"], scalar1=-0.5,
                        scalar2=-0.5 * D * LOG2PI, op0=OP.mult, op1=OP.add)
        STT(out=pl, in0=sld, scalar=-0.5, in1=pl, op0=OP.mult, op1=OP.add)
        V.tensor_tensor(out=osc[:, 2:3], in0=ql, in1=pl, op=OP.subtract)

        nm = scratch.tile([P, 1], f32, tag="fs_nm", name="fs_nm")
        V.tensor_reduce(out=nm, in_=st["qk"], axis=AX.X, op=OP.max, negate=True)
        ex = scratch.tile([P, K], f32, tag="fs_ex", name="fs_ex")
        ssum = scratch.tile([P, 1], f32, tag="fs_ss", name="fs_ss")
        S.activation(out=ex, in_=st["qk"], func=AF.Exp, bias=nm, accum_out=ssum)
        rs = scratch.tile([P, 1], f32, tag="fs_rs", name="fs_rs")
        V.reciprocal(out=rs, in_=ssum)
        oqk = wt([P, K], "oqk")
        V.tensor_scalar(out=oqk, in0=ex, scalar1=rs, scalar2=None, op0=OP.mult)

        nc.sync.dma_start(out=d["o_sample"][r0:r0 + P, :], in_=st["samp"])
        nc.sync.dma_start(out=d["o_sc"][r0:r0 + P, :], in_=osc)
        nc.sync.dma_start(out=d["o_qk"][r0:r0 + P, :], in_=oqk)

    ctx.close()


# --------------------------------------------------------------------------
# host side
# --------------------------------------------------------------------------

def _gumbel_eps():
    import jax
    import jax.numpy as jnp
    cpu = jax.devices("cpu")[0]
    with jax.default_device(cpu):
        u = jax.random.uniform(jax.random.key(1), (N, K),
                               minval=1e-20, maxval=1.0)
        g = -jnp.log(-jnp.log(u))
        ep = jax.random.normal(jax.random.key(2), (N, D))
    return np.asarray(g, np.float32), np.asarray(ep, np.float32)


def make_in_maps(inputs):
    g_all, ep_all = _gumbel_eps()
    z = np.ascontiguousarray(np.asarray(inputs["z_sample"], np.float32))
    ks = np.ascontiguousarray(np.asarray(inputs["k_sample"], np.float32))
    xt = np.ascontiguousarray(np.asarray(inputs["xt"], np.float32))
    mask = np.asarray(inputs["mask"]).astype(np.int32).reshape(N, 1)
    def perm(x, f):
        # [NC(rows), 64, f] -> [NG, P=(two,k), NPAIR, f] per core shard
        x = x.reshape(NCORES, NG, NPAIR, 2, K, f)
        return np.ascontiguousarray(
            x.transpose(0, 1, 3, 4, 2, 5).reshape(NCORES, NG, P, NPAIR, f)
            .astype(np.float16))

    tr = perm(np.asarray(inputs["transition"], np.float32).reshape(N, K, K),
              K)
    Ab = perm(np.asarray(inputs["Ab"], np.float32).reshape(N, K, D * 17),
              D * 17)
    Q = perm(np.asarray(inputs["Q"], np.float32).reshape(N, K, D * D),
             D * D)
    W = np.asarray(inputs["W"], np.float32)
    c = np.asarray(inputs["c"], np.float32)
    wra = np.ascontiguousarray(np.concatenate([W.reshape(K * D, X), c], axis=0))
    qzs = np.ascontiguousarray(np.asarray(inputs["qz_sigma"], np.float32))
    tempv = np.array([[np.float32(inputs["temp"])]], np.float32)

    rep = {
        "fzw0": np.ascontiguousarray(np.asarray(inputs["fz_w0"], np.float32)),
        "fzw1": np.ascontiguousarray(np.asarray(inputs["fz_w1"], np.float32)),
        "fzw2": np.ascontiguousarray(np.asarray(inputs["fz_w2"], np.float32)),
        "ezw0": np.ascontiguousarray(np.asarray(inputs["ez_w0"], np.float32)),
        "ezw1": np.ascontiguousarray(np.asarray(inputs["ez_w1"], np.float32)),
        "ezw2": np.ascontiguousarray(np.asarray(inputs["ez_w2"], np.float32)),
        "wra": wra, "qzs": qzs, "tempv": tempv,
    }
    maps = []
    for ci in range(NCORES):
        lo, hi = ci * NC, (ci + 1) * NC
        m = {
            "z": z[lo:hi], "ks": ks[lo:hi], "xt": xt[lo:hi],
            "maskf": mask[lo:hi], "gn": g_all[lo:hi], "eps": ep_all[lo:hi],
            "trans": tr[ci], "Qp": Q[ci], "Abp": Ab[ci],
        }
        m.update(rep)
        maps.append(m)
    return maps


def assemble(results):
    sample = np.concatenate([np.asarray(r["o_sample"]) for r in results], axis=0)
    sc = np.concatenate([np.asarray(r["o_sc"]) for r in results], axis=0)
    oqk = np.concatenate([np.asarray(r["o_qk"]) for r in results], axis=0)
    out2 = sc[:, 0]
    diw = sc[:, 1]
    qpz = sc[:, 2]
    iwae = np.float32(qpz.astype(np.float32).sum())
    out3 = iwae + diw
    return (np.ascontiguousarray(sample, np.float32),
            np.ascontiguousarray(out2, np.float32),
            np.ascontiguousarray(out3, np.float32),
            np.ascontiguousarray(oqk, np.float32))


def kernel(**inputs):
    global _PROGRAM
    if _PROGRAM is None:
        _PROGRAM = build_program()
    nc, _ = _PROGRAM
    from concourse.bass_utils import run_bass_kernel_spmd
    maps = make_in_maps(inputs)
    res = run_bass_kernel_spmd(nc, maps, list(range(NCORES)))
    return assemble(res.results)


# revision 26
# speedup vs baseline: 1.0676x; 1.0342x over previous
"""Trainium2 Bass kernel for nn_DSLDSCell (moe_routing).

Data-parallel over 8 NeuronCores: each core processes 512 of the 4096 rows.
Heavy per-row k-weighted reductions (transition/Q/Ab contra k_new) run on the
TensorEngine as 2-rows-per-matmul block-diagonal contractions (k=64 on
partitions; 2 consecutive rows fill 128 partitions and are DRAM-contiguous).
Per-row 16x16 factorization uses LDL^T (no sqrt in the recurrence),
vectorized across 128 partitions. LayerNorm rsqrt is DVE-Newton (bit-trick
seed), keeping the ACT engine on only two table sets (gelu / ln+exp).
"""
import math
import sys

for _p in ("/opt/trn_rl_repo",):
    if _p not in sys.path:
        sys.path.insert(0, _p)

import numpy as np

import concourse.bass as bass
import concourse.bacc as bacc
import concourse.tile as tile
from concourse import mybir
from concourse.masks import make_identity

f32 = mybir.dt.float32
i32 = mybir.dt.int32
OP = mybir.AluOpType
AF = mybir.ActivationFunctionType
AX = mybir.AxisListType

N, D, K, X, H = 4096, 16, 64, 128, 256
NCORES = 8
NC = N // NCORES          # rows per core (512)
P = 128                   # partitions
NG = NC // P              # groups per core (4)
NPAIR = P // 2            # pairs per group (64)
NB = D + 2                # rhs columns in the triangular solve
LOG2PI = math.log(2.0 * math.pi)
MAGIC = 0x5F3759DF

f16 = mybir.dt.float16
GELU = "act"                # "act" = ACT Gelu_apprx_tanh; "tanh" = composite
                            # (CoreSim does not implement Gelu)

_PROGRAM = None


# --------------------------------------------------------------------------
# device program
# --------------------------------------------------------------------------

def _patch_ldw_opt():
    """The default walrus invocation passes --enable-ldw-opt=false; our
    stream matmuls are LDWEIGHTS-bound (fp16, 128-column weights), so
    re-enable the fast-weight-load path for this kernel's compiles."""
    import concourse.bass_utils as _bu
    if getattr(_bu, "_ldw_patched", False):
        return
    _orig = _bu.run_command

    def patched(cmd, **kw):
        cmd = ["--enable-ldw-opt=true" if c == "--enable-ldw-opt=false" else c
               for c in cmd]
        return _orig(cmd, **kw)

    _bu.run_command = patched
    _bu._ldw_patched = True


def _patch_act_tables():
    """Force walrus's table-load pass to place Exp/Ln/Square in
    natural_log_exp_and_others and Gelu_apprx_tanh in its own set, so the
    kernel only ever loads two ACT table sets (indices stay file-aligned)."""
    import concourse.bacc as _bacc
    if getattr(_bacc, "_act_tables_patched", False):
        return
    _orig = _bacc.get_activation_tables

    def patched(arch):
        t = _orig(arch)
        keep = {"natural_log_exp_and_others", "gelu_apprx_tanh_and_others"}
        drop = {AF.Exp, AF.Ln, AF.Square, AF.Gelu_apprx_tanh, AF.Tanh}
        for name, fns in t.items():
            if name not in keep:
                for f in drop:
                    fns.discard(f)
        return t

    _bacc.get_activation_tables = patched
    _bacc._act_tables_patched = True


def build_program():
    _patch_act_tables()
    nc = bacc.Bacc("TRN2", debug=False, num_devices=NCORES)

    dd = {}
    def din(name, shape, dt=f32):
        dd[name] = nc.declare_dram_parameter(name, shape, dt, isOutput=False)
    def dout(name, shape):
        dd[name] = nc.declare_dram_parameter(name, shape, f32, isOutput=True)

    din("z", [NC, D]); din("ks", [NC, K]); din("xt", [NC, X])
    din("maskf", [NC, 1], i32); din("gn", [NC, K]); din("eps", [NC, D])
    din("trans", [NG, P, NPAIR, K], f16)
    din("Qp", [NG, P, NPAIR, D * D], f16)
    din("Abp", [NG, P, NPAIR, D * 17], f16)
    din("fzw0", [D + K + X, H]); din("fzw1", [H, H]); din("fzw2", [H, K])
    din("ezw0", [2 * X + K, H]); din("ezw1", [H, H]); din("ezw2", [H, 2 * D])
    din("wra", [K * D + K, X]); din("qzs", [1, D]); din("tempv", [1, 1])
    dout("o_sample", [NC, D]); dout("o_sc", [NC, 4]); dout("o_qk", [NC, K])

    with tile.TileContext(nc) as tc:
        _emit(nc, tc, dd)
    nc.compile()
    return nc, ["o_sample", "o_sc", "o_qk"]


def _bc1(ap, mid, inner):
    """[P, inner] AP -> [P, mid, inner] with zero-stride middle dim."""
    return ap.rearrange("p (a n) -> p a n", a=1).broadcast_to(
        (ap.shape[0], mid, inner))


def _bci(ap, nk, inner):
    """[P, nk] AP -> [P, nk, inner] with zero-stride inner dim."""
    return ap.rearrange("p (k o) -> p k o", o=1).broadcast_to(
        (ap.shape[0], nk, inner))


def _diag(t, n, stride, off=0):
    """[P, n] view of diagonal-ish elements: offset + i*stride."""
    return bass.AP(tensor=t.tensor, offset=t.offset + off,
                   ap=[t.ap[0], [stride, n]])


def _emit(nc, tc, d):
    import contextlib
    ctx = contextlib.ExitStack()
    consts = ctx.enter_context(tc.tile_pool(name="consts", bufs=1))
    work = ctx.enter_context(tc.tile_pool(name="work", bufs=NG + 1))
    scratch = ctx.enter_context(tc.tile_pool(name="scratch", bufs=2))
    stream = ctx.enter_context(tc.tile_pool(name="stream", bufs=2))
    pmm = ctx.enter_context(tc.tile_pool(name="pmm", bufs=2, space="PSUM"))
    ptp = ctx.enter_context(tc.tile_pool(name="ptp", bufs=2, space="PSUM"))
    pstr = ctx.enter_context(tc.tile_pool(name="pstr", bufs=1, space="PSUM"))

    V, T = nc.vector, nc.tensor
    GP = nc.gpsimd
    STT = nc.vector.scalar_tensor_tensor
    _last_act = [None]

    class _SWrap:
        """Chain ACT ops in emission order so the scheduler cannot
        interleave activation-table sets across phases."""
        @staticmethod
        def activation(*a, **kw):
            bi = nc.scalar.activation(*a, **kw)
            if _last_act[0] is not None:
                bass._add_dep_helper(bi.ins, _last_act[0], sync=False,
                                     reason="act-table-order")
            _last_act[0] = bi.ins
            return bi
    S = _SWrap

    # ---------------- constants ----------------
    ident = consts.tile([P, P], f32)
    make_identity(nc, ident)
    magic16 = consts.tile([P, 16], i32)
    V.memset(magic16, MAGIC)
    zero_c = consts.tile([P, P], f32)
    V.memset(zero_c, 0.0)

    def newton_rsqrt(dst, y, tagp, iters=3):
        """dst = 1/sqrt(y); y, dst [P, w] f32 contiguous, w <= 16."""
        w = y.shape[-1]
        hv = scratch.tile([P, w], i32, tag=tagp + "_h", name=tagp + "_h")
        V.tensor_scalar(out=hv, in0=y.bitcast(i32), scalar1=1, scalar2=None,
                        op0=OP.logical_shift_right)
        V.tensor_tensor(out=dst.bitcast(i32), in0=magic16[:, 0:w], in1=hv,
                        op=OP.subtract)
        t = scratch.tile([P, w], f32, tag=tagp + "_t", name=tagp + "_t")
        e = scratch.tile([P, w], f32, tag=tagp + "_e", name=tagp + "_e")
        for _ in range(iters):
            if w == 1:
                STT(out=t, in0=dst, scalar=dst, in1=y, op0=OP.mult, op1=OP.mult)
            else:
                V.tensor_tensor(out=t, in0=dst, in1=dst, op=OP.mult)
                V.tensor_tensor(out=t, in0=t, in1=y, op=OP.mult)
            V.tensor_scalar(out=e, in0=t, scalar1=-0.5, scalar2=1.5,
                            op0=OP.mult, op1=OP.add)
            V.tensor_tensor(out=dst, in0=dst, in1=e, op=OP.mult)

    tempb = consts.tile([P, 1], f32)
    nc.sync.dma_start(out=tempb, in_=bass.AP(
        tensor=d["tempv"], offset=0, ap=[[0, P], [1, 1]]))
    invt = consts.tile([P, 1], f32)
    V.reciprocal(out=invt, in_=tempb)
    ntmp = consts.tile([P, 1], f32)
    V.tensor_scalar(out=ntmp, in0=tempb, scalar1=-1.0, scalar2=None,
                    op0=OP.mult)

    qzs_b = consts.tile([P, D], f32)
    nc.sync.dma_start(out=qzs_b, in_=bass.AP(
        tensor=d["qzs"], offset=0, ap=[[0, P], [1, D]]))
    qzs_cl = consts.tile([P, D], f32)
    V.tensor_scalar(out=qzs_cl, in0=qzs_b, scalar1=-3.0, scalar2=None,
                    op0=OP.max)
    qsig = consts.tile([P, D], f32)
    S.activation(out=qsig, in_=qzs_cl, func=AF.Exp)
    rsig = consts.tile([P, D], f32)
    V.reciprocal(out=rsig, in_=qsig)
    slsig = consts.tile([P, 1], f32)
    V.tensor_reduce(out=slsig, in_=qzs_cl, axis=AX.X, op=OP.add)

    def load_w(name, dn, do):
        chunks = []
        for c in range((dn + P - 1) // P):
            cl = min(P, dn - c * P)
            t = consts.tile([P, do], f32, tag=f"{name}{c}", name=f"{name}{c}")
            nc.sync.dma_start(out=t[:cl, :], in_=d[name][c * P:c * P + cl, :])
            chunks.append((t, cl))
        return chunks

    fzw0 = load_w("fzw0", D + K + X, H)
    fzw1 = load_w("fzw1", H, H)
    fzw2 = load_w("fzw2", H, K)
    ezw0 = load_w("ezw0", 2 * X + K, H)
    ezw1 = load_w("ezw1", H, H)
    ezw2 = load_w("ezw2", H, 2 * D)
    wra = load_w("wra", K * D + K, X)

    def mm(out, lhsT, rhs, **kw):
        T.matmul(out, lhsT, rhs, **kw)

    def layernorm(x, out, tagp):
        st = scratch.tile([P, 6], f32, tag=tagp + "_st", name=tagp + "_st")
        V.bn_stats(out=st, in_=x)
        mv = scratch.tile([P, 2], f32, tag=tagp + "_mv", name=tagp + "_mv")
        V.bn_aggr(out=mv, in_=st)
        ve = scratch.tile([P, 1], f32, tag=tagp + "_ve", name=tagp + "_ve")
        V.tensor_scalar(out=ve, in0=mv[:, 1:2], scalar1=1e-6, scalar2=None,
                        op0=OP.add)
        rst = scratch.tile([P, 1], f32, tag=tagp + "_rs", name=tagp + "_rs")
        newton_rsqrt(rst, ve, tagp, iters=2)
        V.tensor_scalar(out=out, in0=x, scalar1=mv[:, 0:1], scalar2=rst,
                        op0=OP.subtract, op1=OP.mult)

    def dense(x, wchunks, do, gelu, out_sb, tagp, psum_lo=0):
        """out_sb = [gelu](x @ W)[:, psum_lo:psum_lo+width(out_sb)]."""
        xts = []
        for c, (w_, cl) in enumerate(wchunks):
            pt = ptp.tile([P, P], f32, tag="ptp", name="ptp")
            T.transpose(pt[:cl, :], x[:, c * P:c * P + cl], ident)
            xT = scratch.tile([P, P], f32, tag=tagp + f"_xT{c}", name=tagp + f"_xT{c}")
            nc.scalar.copy(out=xT[:cl, :], in_=pt[:cl, :])
            xts.append(xT)
        ph = pmm.tile([P, 512], f32, tag="pmm", name="pmm")
        nchunk = len(wchunks)
        for c, (w_, cl) in enumerate(wchunks):
            mm(ph[:, 0:do], xts[c][:cl, :], w_[:cl, 0:do],
               start=(c == 0), stop=(c == nchunk - 1))
        wo = out_sb.shape[-1]
        if gelu and GELU == "act":
            S.activation(out=out_sb, in_=ph[:, psum_lo:psum_lo + wo],
                         func=AF.Gelu_apprx_tanh)
        elif gelu:
            xs = scratch.tile([P, wo], f32, tag=tagp + "_gx", name=tagp + "_gx")
            V.tensor_copy(out=xs, in_=ph[:, psum_lo:psum_lo + wo])
            t3 = scratch.tile([P, wo], f32, tag=tagp + "_g3", name=tagp + "_g3")
            V.tensor_tensor(out=t3, in0=xs, in1=xs, op=OP.mult)
            V.tensor_tensor(out=t3, in0=t3, in1=xs, op=OP.mult)
            STT(out=t3, in0=t3, scalar=0.044715, in1=xs, op0=OP.mult, op1=OP.add)
            S.activation(out=t3, in_=t3, func=AF.Tanh,
                         scale=0.7978845608028654)
            V.tensor_scalar(out=t3, in0=t3, scalar1=0.5, scalar2=0.5,
                            op0=OP.mult, op1=OP.add)
            V.tensor_tensor(out=out_sb, in0=t3, in1=xs, op=OP.mult)
        else:
            V.tensor_copy(out=out_sb, in_=ph[:, psum_lo:psum_lo + wo])

    def blockdiag_lhsT(src64, tag):
        """src64: [P, K] rows tile -> transpose -> [128,128] block-diag
        columns: col 2t+0 = src row 2t on parts 0:64, col 2t+1 = row 2t+1
        on parts 64:128."""
        pt = ptp.tile([P, P], f32, tag="ptp", name="ptp")
        T.transpose(pt[0:K, :], src64, ident)
        sT = scratch.tile([K, P], f16, tag=tag + "_T", name=tag + "_T")
        nc.scalar.copy(out=sT, in_=pt[0:K, :])
        LA = scratch.tile([P, P], f16, tag=tag + "_LA", name=tag + "_LA")
        GP.memset(LA, 0.0)
        GP.tensor_copy(
            out=LA[0:K, :].rearrange("p (t two) -> p t two", two=2)[:, :, 0:1],
            in_=sT.rearrange("p (t two) -> p t two", two=2)[:, :, 0:1])
        GP.tensor_copy(
            out=LA[K:P, :].rearrange("p (t two) -> p t two", two=2)[:, :, 1:2],
            in_=sT.rearrange("p (t two) -> p t two", two=2)[:, :, 1:2])
        return LA

    G = {}

    def wt(shape, tag):
        return work.tile(shape, f32, tag=tag, name=tag)

    # ============ PHASE A: inputs, fz net (gelu set), transition MMs ========
    for g in range(NG):
        r0 = g * P
        st = {}
        G[g] = st
        in0 = wt([P, D + K + X], "in0")
        nc.sync.dma_start(out=in0[:, 0:D], in_=d["z"][r0:r0 + P, :])
        nc.sync.dma_start(out=in0[:, D:D + K], in_=d["ks"][r0:r0 + P, :])
        nc.sync.dma_start(out=in0[:, D + K:], in_=d["xt"][r0:r0 + P, :])
        st["in0"] = in0
        mfk = work.tile([P, 1], i32, tag="mfk", name="mfk")
        nc.sync.dma_start(out=mfk, in_=d["maskf"][r0:r0 + P, :])
        st["mfk"] = mfk
        gnt = wt([P, K], "gnt")
        nc.sync.dma_start(out=gnt, in_=d["gn"][r0:r0 + P, :])
        st["gnt"] = gnt
        ept = wt([P, D], "ept")
        nc.sync.dma_start(out=ept, in_=d["eps"][r0:r0 + P, :])
        st["ept"] = ept

        # ---- fz net ----
        xh = scratch.tile([P, D + K + X], f32, tag="fz_xh", name="fz_xh")
        layernorm(in0, xh, "fzl0")
        h0 = scratch.tile([P, H], f32, tag="fz_h0", name="fz_h0")
        dense(xh, fzw0, H, True, h0, "fzd0")
        xh1 = scratch.tile([P, H], f32, tag="fz_xh1", name="fz_xh1")
        layernorm(h0, xh1, "fzl1")
        h1 = scratch.tile([P, H], f32, tag="fz_h1", name="fz_h1")
        dense(xh1, fzw1, H, True, h1, "fzd1")
        xh2 = scratch.tile([P, H], f32, tag="fz_xh2", name="fz_xh2")
        layernorm(h1, xh2, "fzl2")
        qk = wt([P, K], "qk")
        dense(xh2, fzw2, K, False, qk, "fzd2")
        st["qk"] = qk

        # ---- transition pair-MMs (data stationary, kn-blockdiag moving) ----
        # out[j, r] per pair -> psum_tT [64, 2t+r]; transpose back afterwards.
        LK = blockdiag_lhsT(in0[:, D:D + K], "kf")
        pairbase = g * NPAIR
        ptT = pstr.tile([K, P], f32, tag="ptT", name="ptT")
        for ch in range(4):                       # 4 chunks of 16 pairs
            trt = stream.tile([P, 16, K], f16, tag="trch", name="trch")
            nc.sync.dma_start(
                out=trt, in_=d["trans"][g, :, ch * 16:(ch + 1) * 16, :])
            for i in range(16):
                pr = ch * 16 + i
                mm(ptT[:, 2 * pr:2 * pr + 2], trt[:, i, :],
                   LK[:, 2 * pr:2 * pr + 2], start=True, stop=True)
        tT_sb = scratch.tile([K, P], f32, tag="tT_sb", name="tT_sb")
        nc.scalar.copy(out=tT_sb, in_=ptT)
        ptb = ptp.tile([P, P], f32, tag="ptp", name="ptp")
        T.transpose(ptb[:, 0:K], tT_sb, ident[0:K, 0:K])
        pkp = wt([P, K], "pkp")
        nc.scalar.copy(out=pkp, in_=ptb[:, 0:K])
        st["pkp"] = pkp

    # ============ PHASE B: ln/exp set — pk_logits, softmaxes, k_new, d_iwae =
    for g in range(NG):
        st = G[g]
        pkl = scratch.tile([P, K], f32, tag="pkl", name="pkl")
        S.activation(out=pkl, in_=st["pkp"], func=AF.Ln)

        gt_n = scratch.tile([P, K], f32, tag="gt_n", name="gt_n")
        V.tensor_scalar(out=gt_n, in0=st["gnt"], scalar1=invt[:, 0:1],
                        scalar2=None, op0=OP.mult)

        def softmax_t(logits, tagp, out_tile):
            sx = scratch.tile([P, K], f32, tag=tagp + "_sx", name=tagp + "_sx")
            STT(out=sx, in0=logits, scalar=invt[:, 0:1], in1=gt_n,
                op0=OP.mult, op1=OP.add)
            nm = scratch.tile([P, 1], f32, tag=tagp + "_nm", name=tagp + "_nm")
            V.tensor_reduce(out=nm, in_=sx, axis=AX.X, op=OP.max, negate=True)
            ex = scratch.tile([P, K], f32, tag=tagp + "_ex", name=tagp + "_ex")
            ssum = scratch.tile([P, 1], f32, tag=tagp + "_ss", name=tagp + "_ss")
            S.activation(out=ex, in_=sx, func=AF.Exp, bias=nm, accum_out=ssum)
            rs = scratch.tile([P, 1], f32, tag=tagp + "_rs", name=tagp + "_rs")
            V.reciprocal(out=rs, in_=ssum)
            V.tensor_scalar(out=out_tile, in0=ex, scalar1=rs, scalar2=None,
                            op0=OP.mult)

        qks = scratch.tile([P, K], f32, tag="qks", name="qks")
        softmax_t(st["qk"], "smq", qks)
        pks = scratch.tile([P, K], f32, tag="pks", name="pks")
        softmax_t(pkl, "smp", pks)

        kn = wt([P, K], "kn")
        V.tensor_copy(out=kn, in_=pks)
        V.copy_predicated(out=kn, mask=st["mfk"][:, 0:1].broadcast_to((P, K)),
                          data=qks)
        st["kn"] = kn

        logx = scratch.tile([P, K], f32, tag="logx", name="logx")
        S.activation(out=logx, in_=kn, func=AF.Ln)

        def lse(logits, tagp, out_t):
            sc = scratch.tile([P, K], f32, tag=tagp + "_sc", name=tagp + "_sc")
            STT(out=sc, in0=logx, scalar=ntmp[:, 0:1], in1=logits,
                op0=OP.mult, op1=OP.add)
            nm = scratch.tile([P, 1], f32, tag=tagp + "_nm", name=tagp + "_nm")
            V.tensor_reduce(out=nm, in_=sc, axis=AX.X, op=OP.max, negate=True)
            ex = scratch.tile([P, K], f32, tag=tagp + "_ex", name=tagp + "_ex")
            sm = scratch.tile([P, 1], f32, tag=tagp + "_sm", name=tagp + "_sm")
            S.activation(out=ex, in_=sc, func=AF.Exp, bias=nm, accum_out=sm)
            ls = scratch.tile([P, 1], f32, tag=tagp + "_ls", name=tagp + "_ls")
            S.activation(out=ls, in_=sm, func=AF.Ln)
            V.tensor_tensor(out=out_t, in0=ls, in1=nm, op=OP.subtract)

        lq = scratch.tile([P, 1], f32, tag="lseq_o", name="lseq_o")
        lse(st["qk"], "lseq", lq)
        lp = scratch.tile([P, 1], f32, tag="lsep_o", name="lsep_o")
        lse(pkl, "lsep", lp)

        df = scratch.tile([P, K], f32, tag="df", name="df")
        V.tensor_tensor(out=df, in0=st["qk"], in1=pkl, op=OP.subtract)
        dsum = scratch.tile([P, 1], f32, tag="dsum", name="dsum")
        V.tensor_reduce(out=dsum, in_=df, axis=AX.X, op=OP.add)
        dl = scratch.tile([P, 1], f32, tag="dl", name="dl")
        V.tensor_tensor(out=dl, in0=lq, in1=lp, op=OP.subtract)
        diw = wt([P, 1], "diw")
        STT(out=diw, in0=dl, scalar=-float(K), in1=dsum,
            op0=OP.mult, op1=OP.add)
        st["diw"] = diw

    # ============ PHASE C: streams, gt, ez net (gelu), LDLT, solves =========
    for g in range(NG):
        st = G[g]
        in0 = st["in0"]
        kn = st["kn"]
        zt = in0[:, 0:D]
        pairbase = g * NPAIR

        LN_ = blockdiag_lhsT(kn, "kn")

        # ---- Q stream: out[de, r] per pair; psum_qT [128, 4t+2h+r] ----
        Qk = wt([P, D * D], "Qk")
        pqT = pstr.tile([P, 4 * NPAIR], f32, tag="pqT", name="pqT")
        for ch in range(4):
            qt = stream.tile([P, 16, D * D], f16, tag="qch", name="qch")
            nc.sync.dma_start(
                out=qt, in_=d["Qp"][g, :, ch * 16:(ch + 1) * 16, :])
            for i in range(16):
                pr = ch * 16 + i
                for h in range(2):
                    mm(pqT[:, 4 * pr + 2 * h:4 * pr + 2 * h + 2],
                       qt[:, i, h * P:(h + 1) * P],
                       LN_[:, 2 * pr:2 * pr + 2], start=True, stop=True)
        qT_sb = scratch.tile([P, 4 * NPAIR], f32, tag="qT_sb", name="qT_sb")
        V.tensor_copy(
            out=qT_sb.rearrange("p (h t r) -> p h t r", h=2, r=2),
            in_=pqT.rearrange("p (t h r) -> p h t r", h=2, r=2))
        for h in range(2):
            ptb = ptp.tile([P, P], f32, tag="ptp", name="ptp")
            T.transpose(ptb[:, :], qT_sb[:, h * P:(h + 1) * P], ident)
            nc.scalar.copy(out=Qk[:, h * P:(h + 1) * P], in_=ptb[:, :])

        # ---- Ab stream: 3 de-chunks (128,128,16); psum_aT [128, 6t+2h+r] ----
        Abk = wt([P, D * 17], "Abk")
        paT = pstr.tile([P, 6 * NPAIR], f32, tag="paT", name="paT")
        for ch in range(4):
            at = stream.tile([P, 16, D * 17], f16, tag="abch", name="abch")
            nc.sync.dma_start(
                out=at, in_=d["Abp"][g, :, ch * 16:(ch + 1) * 16, :])
            for i in range(16):
                pr = ch * 16 + i
                for h in range(3):
                    cw = P if h < 2 else D * 17 - 2 * P
                    mm(paT[0:cw, 6 * pr + 2 * h:6 * pr + 2 * h + 2],
                       at[:, i, h * P:h * P + cw],
                       LN_[:, 2 * pr:2 * pr + 2], start=True, stop=True)
        aT_sb = scratch.tile([P, 6 * NPAIR], f32, tag="aT_sb", name="aT_sb")
        V.tensor_copy(
            out=aT_sb[:, 0:2 * P].rearrange("p (h t r) -> p h t r", h=2, r=2),
            in_=paT.rearrange("p (t hh r) -> p t hh r", hh=3, r=2)[:, :, 0:2, :]
            .rearrange("p t h r -> p h t r"))
        V.tensor_copy(
            out=aT_sb[0:16, 2 * P:2 * P + P].rearrange("p (t r) -> p t r", r=2),
            in_=paT[0:16, :].rearrange("p (t hh r) -> p t hh r", hh=3, r=2)
            [:, :, 2:3, :].rearrange("p t h r -> p t (h r)"))
        for h in range(3):
            cw = P if h < 2 else D * 17 - 2 * P
            ptb = ptp.tile([P, P], f32, tag="ptp", name="ptp")
            T.transpose(ptb[:, 0:cw], aT_sb[0:cw, h * P:h * P + P],
                        ident[0:cw, 0:cw])
            nc.scalar.copy(out=Abk[:, h * P:h * P + cw], in_=ptb[:, 0:cw])

        # ---- pz_mu = z @ A + b ----
        AbkV = Abk.rearrange("p (i e) -> p i e", e=17)
        tpm = scratch.tile([P, D * D], f32, tag="tpm", name="tpm")
        GP.tensor_tensor(out=tpm.rearrange("p (e i) -> p e i", e=D),
                         in0=AbkV[:, :, 0:16].rearrange("p i e -> p e i"),
                         in1=_bc1(zt, D, D), op=OP.mult)
        pzA = scratch.tile([P, D], f32, tag="pzA", name="pzA")
        V.tensor_reduce(out=pzA, in_=tpm.rearrange("p (e i) -> p e i", e=D),
                        axis=AX.X, op=OP.add)
        pzmu = wt([P, D], "pzmu")
        V.tensor_tensor(out=pzmu, in0=pzA, in1=_diag(Abk, D, 17, off=16),
                        op=OP.add)
        st["pzmu"] = pzmu

        # ---- gt = [outer(kn, z) | kn] @ wra ----
        Y = scratch.tile([P, K * D + K], f32, tag="Y", name="Y")
        V.tensor_tensor(out=Y[:, 0:K * D].rearrange("p (k i) -> p k i", k=K),
                        in0=_bci(kn, K, D), in1=_bc1(zt, K, D), op=OP.mult)
        GP.tensor_copy(out=Y[:, K * D:], in_=kn)
        yts = []
        for c, (w_, cl) in enumerate(wra):
            pt = ptp.tile([P, P], f32, tag="ptp", name="ptp")
            T.transpose(pt[:cl, :], Y[:, c * P:c * P + cl], ident)
            yT = scratch.tile([P, P], f32, tag=f"yT{c}", name=f"yT{c}")
            nc.scalar.copy(out=yT[:cl, :], in_=pt[:cl, :])
            yts.append(yT)
        pg = pmm.tile([P, 512], f32, tag="pmm", name="pmm")
        for c, (w_, cl) in enumerate(wra):
            mm(pg[:, 0:X], yts[c][:cl, :], w_[:cl, 0:X],
               start=(c == 0), stop=(c == len(wra) - 1))

        # ---- ez net ----
        ein = scratch.tile([P, 2 * X + K], f32, tag="ein", name="ein")
        nc.scalar.copy(out=ein[:, 0:X], in_=pg[:, 0:X])
        GP.tensor_copy(out=ein[:, X:X + K], in_=kn)
        GP.tensor_copy(out=ein[:, X + K:], in_=in0[:, D + K:])
        exh = scratch.tile([P, 2 * X + K], f32, tag="ez_xh", name="ez_xh")
        layernorm(ein, exh, "ezl0")
        eh0 = scratch.tile([P, H], f32, tag="ez_h0", name="ez_h0")
        dense(exh, ezw0, H, True, eh0, "ezd0")
        exh1 = scratch.tile([P, H], f32, tag="ez_xh1", name="ez_xh1")
        layernorm(eh0, exh1, "ezl1")
        eh1 = scratch.tile([P, H], f32, tag="ez_h1", name="ez_h1")
        dense(exh1, ezw1, H, True, eh1, "ezd1")
        exh2 = scratch.tile([P, H], f32, tag="ez_xh2", name="ez_xh2")
        layernorm(eh1, exh2, "ezl2")
        qzmu = wt([P, D], "qzmu")
        dense(exh2, ezw2, 2 * D, False, qzmu, "ezd2", psum_lo=0)
        st["qzmu"] = qzmu

        # ---- LDL^T of Qk:  Qk = Lu diag(D) Lu^T ----
        Lu = wt([P, D * D], "Lu")           # unit-lower, strict lower stored
        GP.memset(Lu, 0.0)
        LDt = wt([P, D * D], "LDt")         # LD[i,j] = Lu[i,j]*D_j; diag = D
        Dinv = wt([P, D], "Dinv")
        LuV = Lu.rearrange("p (i j) -> p i j", j=D)
        LDV = LDt.rearrange("p (i j) -> p i j", j=D)
        QkV = Qk.rearrange("p (i j) -> p i j", j=D)
        V.tensor_copy(out=LDV[:, :, 0:1], in_=QkV[:, :, 0:1])
        V.reciprocal(out=Dinv[:, 0:1], in_=LDt[:, 0:1])
        V.tensor_scalar(out=LuV[:, 1:, 0:1], in0=LDV[:, 1:, 0:1],
                        scalar1=Dinv[:, 0:1], scalar2=None, op0=OP.mult)
        for j in range(1, D):
            nr = D - j
            tmpd = scratch.tile([P, nr * j], f32, tag="ch_tmp", name="ch_tmp")
            V.tensor_tensor(out=tmpd.rearrange("p (i t) -> p i t", t=j),
                            in0=LuV[:, j:, 0:j],
                            in1=_bc1(LDt[:, j * D:j * D + j], nr, j),
                            op=OP.mult)
            sd = scratch.tile([P, nr], f32, tag="ch_sd", name="ch_sd")
            V.tensor_reduce(out=sd, in_=tmpd.rearrange("p (i t) -> p i t", t=j),
                            axis=AX.X, op=OP.add)
            V.tensor_tensor(out=LDV[:, j:, j:j + 1], in0=QkV[:, j:, j:j + 1],
                            in1=sd.rearrange("p (i o) -> p i o", o=1),
                            op=OP.subtract)
            V.reciprocal(out=Dinv[:, j:j + 1],
                         in_=LDt[:, j * D + j:j * D + j + 1])
            if j < D - 1:
                V.tensor_scalar(out=LuV[:, j + 1:, j:j + 1],
                                in0=LDV[:, j + 1:, j:j + 1],
                                scalar1=Dinv[:, j:j + 1], scalar2=None,
                                op0=OP.mult)
        st["LDt"], st["Dinv"] = LDt, Dinv

        Dc = scratch.tile([P, D], f32, tag="Dc", name="Dc")
        V.tensor_copy(out=Dc, in_=_diag(LDt, D, D + 1))
        rD = scratch.tile([P, D], f32, tag="rD", name="rD")
        newton_rsqrt(rD, Dc, "rD")
        sqD = scratch.tile([P, D], f32, tag="sqD", name="sqD")
        V.tensor_tensor(out=sqD, in0=Dc, in1=rD, op=OP.mult)

        # ---- sample ----
        u = scratch.tile([P, D], f32, tag="u_t", name="u_t")
        V.tensor_tensor(out=u, in0=sqD, in1=st["ept"], op=OP.mult)
        tl = scratch.tile([P, D * D], f32, tag="tl", name="tl")
        V.tensor_tensor(out=tl.rearrange("p (i t) -> p i t", t=D),
                        in0=LuV, in1=_bc1(u, D, D), op=OP.mult)
        Lu0 = scratch.tile([P, D], f32, tag="Lu0", name="Lu0")
        V.tensor_reduce(out=Lu0, in_=tl.rearrange("p (i t) -> p i t", t=D),
                        axis=AX.X, op=OP.add)
        Leps = scratch.tile([P, D], f32, tag="Leps", name="Leps")
        V.tensor_tensor(out=Leps, in0=Lu0, in1=u, op=OP.add)
        pzs = scratch.tile([P, D], f32, tag="pzs", name="pzs")
        V.tensor_tensor(out=pzs, in0=Leps, in1=pzmu, op=OP.add)
        V.tensor_scalar(out=pzs, in0=pzs, scalar1=100.0, scalar2=-100.0,
                        op0=OP.min, op1=OP.max)
        qse = scratch.tile([P, D], f32, tag="qse", name="qse")
        V.tensor_tensor(out=qse, in0=qsig, in1=st["ept"], op=OP.mult)
        V.tensor_tensor(out=qse, in0=qse, in1=st["qzmu"], op=OP.add)
        samp = wt([P, D], "samp")
        GP.tensor_copy(out=samp, in_=pzs)
        V.copy_predicated(out=samp, mask=st["mfk"][:, 0:1].broadcast_to((P, D)),
                          data=qse)
        st["samp"] = samp

        # ---- B build + unit-lower forward substitution (in place) ----
        B = wt([P, D * NB], "Bx")
        GP.memset(B, 0.0)
        GP.tensor_copy(out=_diag(B, D, NB + 1), in_=qsig)
        BV = B.rearrange("p (i c) -> p i c", c=NB)
        V.tensor_tensor(out=BV[:, :, D:D + 1],
                        in0=st["qzmu"].rearrange("p (i o) -> p i o", o=1),
                        in1=pzmu.rearrange("p (i o) -> p i o", o=1),
                        op=OP.subtract)
        V.tensor_tensor(out=BV[:, :, D + 1:D + 2],
                        in0=samp.rearrange("p (i o) -> p i o", o=1),
                        in1=pzmu.rearrange("p (i o) -> p i o", o=1),
                        op=OP.subtract)
        BP = B.rearrange("p (t c) -> p c t", c=NB)      # [P, NB, D]
        for i in range(1, D):
            tms = scratch.tile([P, NB * i], f32, tag="sb_tm", name="sb_tm")
            V.tensor_tensor(out=tms.rearrange("p (c t) -> p c t", t=i),
                            in0=BP[:, :, 0:i],
                            in1=_bc1(Lu[:, i * D:i * D + i], NB, i),
                            op=OP.mult)
            sv = scratch.tile([P, NB], f32, tag="sb_sv", name="sb_sv")
            V.tensor_reduce(out=sv, in_=tms.rearrange("p (c t) -> p c t", t=i),
                            axis=AX.X, op=OP.add)
            V.tensor_tensor(out=B[:, i * NB:(i + 1) * NB],
                            in0=B[:, i * NB:(i + 1) * NB], in1=sv,
                            op=OP.subtract)

        # ---- weighted norms:  sum_i row_i^2 * Dinv_i ----
        sqX = scratch.tile([P, D * NB], f32, tag="sqX", name="sqX")
        GP.tensor_tensor(out=sqX, in0=B, in1=B, op=OP.mult)
        V.tensor_tensor(out=sqX.rearrange("p (i c) -> p i c", c=NB),
                        in0=sqX.rearrange("p (i c) -> p i c", c=NB),
                        in1=_bci(Dinv, D, NB), op=OP.mult)
        sqP = sqX.rearrange("p (i c) -> p c i", c=NB)    # [P, NB, D]
        tF = wt([P, 1], "tF")
        V.tensor_reduce(out=tF,
                        in_=sqX.rearrange("p (i c) -> p i c", c=NB)[:, :, 0:D],
                        axis=AX.XY, op=OP.add)
        tY = wt([P, 1], "tY")
        V.tensor_reduce(out=tY, in_=sqP[:, D:D + 1, :], axis=AX.X, op=OP.add)
        tW = wt([P, 1], "tW")
        V.tensor_reduce(out=tW, in_=sqP[:, D + 1:D + 2, :], axis=AX.X, op=OP.add)
        st["tF"], st["tY"], st["tW"] = tF, tY, tW

    # ============ PHASE D: ln/exp set — logdet, lps, kl, final softmax ======
    for g in range(NG):
        st = G[g]
        r0 = g * P
        osc = wt([P, 4], "osc")
        GP.memset(osc, 0.0)

        jk16 = scratch.tile([P, D], f32, tag="jk16", name="jk16")
        sld = scratch.tile([P, 1], f32, tag="sld", name="sld")       # logdet_p = sum ln D
        S.activation(out=jk16, in_=_diag(st["LDt"], D, D + 1),
                     func=AF.Ln, accum_out=sld)

        klA = scratch.tile([P, 1], f32, tag="klA", name="klA")
        V.tensor_tensor(out=klA, in0=st["tF"], in1=st["tY"], op=OP.add)
        klB = scratch.tile([P, 1], f32, tag="klB", name="klB")
        V.tensor_scalar(out=klB, in0=klA, scalar1=0.5, scalar2=-0.5 * D,
                        op0=OP.mult, op1=OP.add)
        klC = scratch.tile([P, 1], f32, tag="klC", name="klC")
        STT(out=klC, in0=sld, scalar=0.5, in1=klB, op0=OP.mult, op1=OP.add)
        klf = scratch.tile([P, 1], f32, tag="klf", name="klf")
        V.tensor_tensor(out=klf, in0=klC, in1=slsig, op=OP.subtract)
        V.tensor_tensor(out=osc[:, 0:1], in0=klf, in1=st["diw"], op=OP.add)
        GP.tensor_copy(out=osc[:, 1:2], in_=st["diw"])

        dq = scratch.tile([P, D], f32, tag="dq", name="dq")
        V.tensor_tensor(out=dq, in0=st["samp"], in1=st["qzmu"], op=OP.subtract)
        V.tensor_tensor(out=dq, in0=dq, in1=rsig, op=OP.mult)
        jkq = scratch.tile([P, D], f32, tag="jkq", name="jkq")
        sq2 = scratch.tile([P, 1], f32, tag="sq2", name="sq2")
        S.activation(out=jkq, in_=dq, func=AF.Square, accum_out=sq2)
        ql = scratch.tile([P, 1], f32, tag="ql", name="ql")
        V.tensor_scalar(out=ql, in0=sq2, scalar1=-0.5,
                        scalar2=-0.5 * D * LOG2PI, op0=OP.mult, op1=OP.add)
        V.tensor_tensor(out=ql, in0=ql, in1=slsig, op=OP.subtract)
        pl = scratch.tile([P, 1], f32, tag="pl", name="pl")
        V.tensor_scalar(out=pl, in0=st["tW"], scalar1=-0.5,
                        scalar2=-0.5 * D * LOG2PI, op0=OP.mult, op1=OP.add)
        STT(out=pl, in0=sld, scalar=-0.5, in1=pl, op0=OP.mult, op1=OP.add)
        V.tensor_tensor(out=osc[:, 2:3], in0=ql, in1=pl, op=OP.subtract)

        nm = scratch.tile([P, 1], f32, tag="fs_nm", name="fs_nm")
        V.tensor_reduce(out=nm, in_=st["qk"], axis=AX.X, op=OP.max, negate=True)
        ex = scratch.tile([P, K], f32, tag="fs_ex", name="fs_ex")
        ssum = scratch.tile([P, 1], f32, tag="fs_ss", name="fs_ss")
        S.activation(out=ex, in_=st["qk"], func=AF.Exp, bias=nm, accum_out=ssum)
        rs = scratch.tile([P, 1], f32, tag="fs_rs", name="fs_rs")
        V.reciprocal(out=rs, in_=ssum)
        oqk = wt([P, K], "oqk")
        V.tensor_scalar(out=oqk, in0=ex, scalar1=rs, scalar2=None, op0=OP.mult)

        nc.sync.dma_start(out=d["o_sample"][r0:r0 + P, :], in_=st["samp"])
        nc.sync.dma_start(out=d["o_sc"][r0:r0 + P, :], in_=osc)
        nc.sync.dma_start(out=d["o_qk"][r0:r0 + P, :], in_=oqk)

    ctx.close()


# --------------------------------------------------------------------------
# host side
# --------------------------------------------------------------------------

def _gumbel_eps():
    import jax
    import jax.numpy as jnp
    cpu = jax.devices("cpu")[0]
    with jax.default_device(cpu):
        u = jax.random.uniform(jax.random.key(1), (N, K),
                               minval=1e-20, maxval=1.0)
        g = -jnp.log(-jnp.log(u))
        ep = jax.random.normal(jax.random.key(2), (N, D))
    return np.asarray(g, np.float32), np.asarray(ep, np.float32)


def make_in_maps(inputs):
    g_all, ep_all = _gumbel_eps()
    z = np.ascontiguousarray(np.asarray(inputs["z_sample"], np.float32))
    ks = np.ascontiguousarray(np.asarray(inputs["k_sample"], np.float32))
    xt = np.ascontiguousarray(np.asarray(inputs["xt"], np.float32))
    mask = np.asarray(inputs["mask"]).astype(np.int32).reshape(N, 1)
    def perm(x, f):
        # [NC(rows), 64, f] -> [NG, P=(two,k), NPAIR, f] per core shard
        x = x.reshape(NCORES, NG, NPAIR, 2, K, f)
        return np.ascontiguousarray(
            x.transpose(0, 1, 3, 4, 2, 5).reshape(NCORES, NG, P, NPAIR, f)
            .astype(np.float16))

    tr = perm(np.asarray(inputs["transition"], np.float32).reshape(N, K, K),
              K)
    Ab = perm(np.asarray(inputs["Ab"], np.float32).reshape(N, K, D * 17),
              D * 17)
    Q = perm(np.asarray(inputs["Q"], np.float32).reshape(N, K, D * D),
             D * D)
    W = np.asarray(inputs["W"], np.float32)
    c = np.asarray(inputs["c"], np.float32)
    wra = np.ascontiguousarray(np.concatenate([W.reshape(K * D, X), c], axis=0))
    qzs = np.ascontiguousarray(np.asarray(inputs["qz_sigma"], np.float32))
    tempv = np.array([[np.float32(inputs["temp"])]], np.float32)

    rep = {
        "fzw0": np.ascontiguousarray(np.asarray(inputs["fz_w0"], np.float32)),
        "fzw1": np.ascontiguousarray(np.asarray(inputs["fz_w1"], np.float32)),
        "fzw2": np.ascontiguousarray(np.asarray(inputs["fz_w2"], np.float32)),
        "ezw0": np.ascontiguousarray(np.asarray(inputs["ez_w0"], np.float32)),
        "ezw1": np.ascontiguousarray(np.asarray(inputs["ez_w1"], np.float32)),
        "ezw2": np.ascontiguousarray(np.asarray(inputs["ez_w2"], np.float32)),
        "wra": wra, "qzs": qzs, "tempv": tempv,
    }
    maps = []
    for ci in range(NCORES):
        lo, hi = ci * NC, (ci + 1) * NC
        m = {
            "z": z[lo:hi], "ks": ks[lo:hi], "xt": xt[lo:hi],
            "maskf": mask[lo:hi], "gn": g_all[lo:hi], "eps": ep_all[lo:hi],
            "trans": tr[ci], "Qp": Q[ci], "Abp": Ab[ci],
        }
        m.update(rep)
        maps.append(m)
    return maps


def assemble(results):
    sample = np.concatenate([np.asarray(r["o_sample"]) for r in results], axis=0)
    sc = np.concatenate([np.asarray(r["o_sc"]) for r in results], axis=0)
    oqk = np.concatenate([np.asarray(r["o_qk"]) for r in results], axis=0)
    out2 = sc[:, 0]
    diw = sc[:, 1]
    qpz = sc[:, 2]
    iwae = np.float32(qpz.astype(np.float32).sum())
    out3 = iwae + diw
    return (np.ascontiguousarray(sample, np.float32),
            np.ascontiguousarray(out2, np.float32),
            np.ascontiguousarray(out3, np.float32),
            np.ascontiguousarray(oqk, np.float32))


def kernel(**inputs):
    global _PROGRAM
    if _PROGRAM is None:
        _PROGRAM = build_program()
    nc, _ = _PROGRAM
    from concourse.bass_utils import run_bass_kernel_spmd
    maps = make_in_maps(inputs)
    res = run_bass_kernel_spmd(nc, maps, list(range(NCORES)))
    return assemble(res.results)
